# revision 24
# baseline (speedup 1.0000x reference)
"""Multi-head attention (B=2, N=2048, D=1024, H=16) on 8 TRN2 NeuronCores.

Sharding: core c handles batch b=c//4 and head group hg=c%4 (4 heads of 16).
Each core computes QKV for its heads, materialized attention, and a partial
projection (proj row-split over heads); the host sums 4 partials per batch
and adds proj bias.  No device collectives.

v2 schedule, engineered to the PE roofline (~136.5us of moving-row time):
  - chunk-granular input DMA (one descriptor-batch per x chunk) spread over
    4 queues so the first score matmul fires at ~6us
  - hp-interleaved unit order (0,0),(1,0),(0,1),(1,1),... so projection
    work for chunk cc unlocks right after unit (1,cc) and spreads forward
  - exp split: most kb-slabs on ACT (hardware Exp), kbs in DVE_KBS per unit
    computed on the Vector engine with a Schraudolph fast-exp (scores*A+B
    -> int16 -> bitcast bf16), keeping ACT under the PE floor
  - V bias-add + normalize-mul + proj-psum drain on Pool, score bias +
    AV-psum drain + reciprocal on Vector: no engine above ~60% of the span
  - PSUM: 4 banks score double-buffer, 2 banks AV accumulators, 2 banks
    shared QKV/V/proj staging (prologue QK groups borrow the score banks)
"""

import numpy as np

B, N, DIM, H, DH = 2, 2048, 1024, 16, 64
SCALE = DH ** -0.5
NCORE = 8
HPC = 4            # heads per core
F = HPC * DH       # 256 features per core-headgroup
CH = 512           # token chunk (matmul moving free dim)
NCH = N // CH      # 4
KT = DIM // 128    # 8 k-tiles over model dim
TB = N // 128      # 16 token blocks
DVE_KBS = (1, 3, 6, 9, 11, 14)  # kb slabs per unit whose exp runs on DVE (fast-exp)
EXP_A = SCALE * (2.0 ** 7) / float(np.log(2.0))   # schraudolph multiplier
EXP_B = 127.0 * 128.0 - 7.0                        # schraudolph bias (c=7)
_cache = {}


def _build():
    from contextlib import ExitStack

    import concourse.mybir as mybir
    from concourse import bacc
    from concourse.tile import TileContext

    f32 = mybir.dt.float32
    bf16 = mybir.dt.bfloat16
    i16 = mybir.dt.int16
    nc = bacc.Bacc("TRN2", target_bir_lowering=False)

    xt_d = nc.declare_dram_parameter("xt", [DIM, N], bf16, isOutput=False)
    wqk_d = nc.declare_dram_parameter("wqk", [DIM, 2 * F], bf16, isOutput=False)
    wv_d = nc.declare_dram_parameter("wv", [DIM, F], bf16, isOutput=False)
    bqk_d = nc.declare_dram_parameter("bqk", [2 * F], f32, isOutput=False)
    bv_d = nc.declare_dram_parameter("bv", [F], f32, isOutput=False)
    pw_d = nc.declare_dram_parameter("pw", [F, DIM], bf16, isOutput=False)
    out_d = nc.declare_dram_parameter("out", [DIM, N], f32, isOutput=True)
    rscr = nc.dram_tensor("rscr", [2, NCH, 2 * CH], f32)

    # chunk-major views: one DMA delivers [128, 8, *] (all 8 k-tiles)
    xt_r = xt_d.ap().rearrange("(t p) n -> p t n", p=128)
    wqk_r = wqk_d.ap().rearrange("(t p) m -> p t m", p=128)
    wv_r = wv_d.ap().rearrange("(t p) m -> p t m", p=128)
    pw_r = pw_d.ap().rearrange("(t p) m -> t p m", p=128)
    out_r = out_d.ap().rearrange("(t p) n -> t p n", p=128)

    with TileContext(nc) as tc, ExitStack() as st:
        consts = st.enter_context(tc.tile_pool(name="consts", bufs=1))
        qkp = st.enter_context(tc.tile_pool(name="qkp", bufs=1))
        vtp = st.enter_context(tc.tile_pool(name="vtp", bufs=1))
        otp = st.enter_context(tc.tile_pool(name="otp", bufs=1))
        ep = st.enter_context(tc.tile_pool(name="ep", bufs=2))
        recp = st.enter_context(tc.tile_pool(name="recp", bufs=2))
        outs = st.enter_context(tc.tile_pool(name="outs", bufs=3))
        stgp = st.enter_context(tc.tile_pool(name="stgp", bufs=2))
        xw = st.enter_context(tc.tile_pool(name="xw", bufs=1))
        ps_mm = st.enter_context(tc.tile_pool(name="ps_mm", bufs=1, space="PSUM"))
        ps_s = st.enter_context(tc.tile_pool(name="ps_s", bufs=2, space="PSUM"))
        ps_av = st.enter_context(tc.tile_pool(name="ps_av", bufs=3, space="PSUM"))

        # ---- constant + weight tiles -------------------------------------
        bqk_sb = consts.tile([128, 2 * F // 128], f32)
        bv_sb = consts.tile([128, F], f32)
        # host supplies wqk with columns reordered to [k01 | q01 | k23 | q23]
        wkq01_sb = xw.tile([128, KT, F], bf16)
        wkq23_sb = xw.tile([128, KT, F], bf16)
        wv_sb = xw.tile([128, KT, F], bf16)
        pw_sb = [consts.tile([128, DIM], bf16, tag=f"pw{t}", name=f"pw{t}")
                 for t in range(2)]
        xt_sb = [xw.tile([128, KT, CH], bf16, tag=f"x{ch}", name=f"x{ch}")
                 for ch in range(NCH)]

        # DMA plan (issue ~0.6us fixed, ~150GB/s per ring, keep elements
        # >=512B).  Ring loads: sync: kq01, x0b, x1 | scalar: wv, kq23,
        # x3, pw | gpsimd: x0a, biases, x2.  k01+q01+x0 gate the first
        # score matmul.
        nc.sync.dma_start(out=wkq01_sb[:, :4, :], in_=wqk_r[:, :4, :F])
        nc.gpsimd.dma_start(out=xt_sb[0][:, :4, :], in_=xt_r[:, :4, 0 * CH:1 * CH])
        nc.scalar.dma_start(out=wv_sb, in_=wv_r)
        nc.sync.dma_start(out=wkq01_sb[:, 4:, :], in_=wqk_r[:, 4:, :F])
        nc.gpsimd.dma_start(out=bqk_sb, in_=bqk_d.ap().rearrange("(f p) -> p f", p=128))
        nc.gpsimd.dma_start(out=bv_sb, in_=bv_d.ap().partition_broadcast(128))
        nc.gpsimd.dma_start(out=xt_sb[0][:, 4:, :], in_=xt_r[:, 4:, 0 * CH:1 * CH])
        nc.scalar.dma_start(out=wkq23_sb, in_=wqk_r[:, :, F:])
        nc.sync.dma_start(out=xt_sb[1], in_=xt_r[:, :, 1 * CH:2 * CH])
        nc.gpsimd.dma_start(out=xt_sb[2], in_=xt_r[:, :, 2 * CH:3 * CH])
        nc.scalar.dma_start(out=xt_sb[3], in_=xt_r[:, :, 3 * CH:4 * CH])
        for t in range(2):
            nc.scalar.dma_start(out=pw_sb[t], in_=pw_r[t])

        # ---- working tiles ----------------------------------------------
        qk_sb = [[qkp.tile([128, CH], bf16, tag=f"qk{fb}_{ch}", name=f"qk{fb}_{ch}")
                  for ch in range(NCH)] for fb in range(4)]
        vt_sb = [vtp.tile([128, HPC, DH + 1], bf16, tag=f"vt{tb}", name=f"vt{tb}")
                 for tb in range(TB)]
        ot_sb = [[otp.tile([128, CH], bf16, tag=f"ot{t}_{ch}", name=f"ot{t}_{ch}")
                  for ch in range(NCH)] for t in range(2)]

        def emit_qk_group(fb, ch, pool):
            # fb: 0=q01 1=q23 2=k01 3=k23; host column order [k01 q01 k23 q23]
            w = wkq01_sb if fb % 2 == 0 else wkq23_sb
            wo = 0 if fb >= 2 else 128
            ps = pool.tile([128, CH], f32, tag=pool.name.startswith("ps_s") and "sp" or "mm",
                           name=f"qkg{fb}_{ch}")
            for t in range(KT):
                nc.tensor.matmul(
                    ps,
                    w[:, t, wo:wo + 128],
                    xt_sb[ch][:, t, :],
                    start=(t == 0), stop=(t == KT - 1),
                )
            nc.vector.tensor_scalar_add(
                out=qk_sb[fb][ch], in0=ps, scalar1=bqk_sb[:, fb:fb + 1],
            )

        def emit_v_group(tb):
            # prologue-only: borrows the (still unused) AV psum slots
            ps = ps_av.tile([128, F], f32, tag="av", name=f"vg{tb}")
            ch, blk = tb // 4, tb % 4
            for t in range(KT):
                nc.tensor.matmul(
                    ps,
                    xt_sb[ch][:, t, blk * 128:(blk + 1) * 128],
                    wv_sb[:, t, :],
                    start=(t == 0), stop=(t == KT - 1),
                )
            nc.vector.tensor_add(
                out=vt_sb[tb][:, :, :DH],
                in0=ps.rearrange("p (h d) -> p h d", h=HPC),
                in1=bv_sb.rearrange("p (h d) -> p h d", h=HPC),
            )
            nc.vector.memset(vt_sb[tb][:, :, DH:], 1.0)

        # ---- attention units: unit = (head-pair hp, chunk cc) ------------
        units = [(hp, cc) for cc in range(NCH) for hp in (0, 1)]
        et_store = {}

        def q_slice(h, cc):
            return qk_sb[h // 2][cc][(h % 2) * 64:(h % 2) * 64 + 64, :]

        def k_slice(h, kb):
            t = qk_sb[2 + h // 2][kb // 4]
            return t[(h % 2) * 64:(h % 2) * 64 + 64, (kb % 4) * 128:(kb % 4 + 1) * 128]

        def emit_s(u, kb):
            hp, cc = u
            sp = ps_s.tile([128, 2 * CH], f32, tag="sp", name=f"sp{hp}_{cc}_{kb}")
            for j in range(2):
                h = 2 * hp + j
                nc.tensor.matmul(
                    sp[:, j * CH:(j + 1) * CH],
                    k_slice(h, kb),
                    q_slice(h, cc),
                    start=True, stop=True,
                )
            e = ep.tile([128, 2 * CH], bf16, tag=f"e{kb}", name=f"e{hp}_{cc}_{kb}")
            if kb in DVE_KBS:
                nc.vector.tensor_scalar(
                    out=e.bitcast(i16), in0=sp,
                    scalar1=float(EXP_A), scalar2=float(EXP_B),
                    op0=mybir.AluOpType.mult, op1=mybir.AluOpType.add,
                )
            else:
                nc.scalar.activation(
                    out=e, in_=sp,
                    func=mybir.ActivationFunctionType.Exp, scale=SCALE,
                )
            et_store[u][kb] = e

        dma_rr = [nc.sync, nc.gpsimd, nc.scalar]

        def proj_ops(fb, cc):
            # projection group split into two single-matmul filler ops; the
            # psum drain runs on the (slack) ACT engine
            state = {}

            def op1():
                ps = ps_mm.tile([128, CH], f32, tag="mm", name=f"pj{fb}_{cc}")
                state["ps"] = ps
                nc.tensor.matmul(ps, pw_sb[0][:, fb * 128:(fb + 1) * 128],
                                 ot_sb[0][cc], start=True, stop=False)

            def op2():
                ps = state["ps"]
                nc.tensor.matmul(ps, pw_sb[1][:, fb * 128:(fb + 1) * 128],
                                 ot_sb[1][cc], start=False, stop=True)
                os = outs.tile([128, CH], f32, tag="os", name=f"os{fb}_{cc}")
                nc.scalar.copy(out=os, in_=ps)
                dma_rr[fb % 3].dma_start(
                    out=out_r[fb][:, cc * CH:(cc + 1) * CH], in_=os
                )

            return [("proj", op1), ("proj", op2)]

        def qk_ops(fb, ch):
            # QK group as 8 single-matmul ops + bias drain on the last
            w = wkq01_sb if fb % 2 == 0 else wkq23_sb
            wo = 0 if fb >= 2 else 128
            state = {}
            ops = []

            def mk(t):
                def op():
                    if t == 0:
                        state["ps"] = ps_mm.tile([128, CH], f32, tag="mm",
                                                 name=f"qkg{fb}_{ch}")
                    nc.tensor.matmul(state["ps"], w[:, t, wo:wo + 128],
                                     xt_sb[ch][:, t, :],
                                     start=(t == 0), stop=(t == KT - 1))
                    if t == KT - 1:
                        nc.vector.tensor_scalar_add(
                            out=qk_sb[fb][ch], in0=state["ps"],
                            scalar1=bqk_sb[:, fb:fb + 1])
                return op

            for t in range(KT):
                ops.append(("qk", mk(t)))
            return ops

        # ---- prologue: k01/q01 chunk0, then unit-0 scores + V fillers ----
        emit_qk_group(2, 0, ps_s)   # k01 c0 (borrows a score bank)
        emit_qk_group(0, 0, ps_s)   # q01 c0
        et_store[units[0]] = [None] * TB
        # per-kb filler: one V group per slab; k01(c+1) before S needs it;
        # k23/q23 chunk0 early so unit (1,0)'s score stream can start.
        pro_fill = {1: [(3, 0, ps_mm)], 3: [(2, 1, ps_mm)], 5: [(1, 0, ps_mm)],
                    7: [(2, 2, ps_mm)], 11: [(2, 3, ps_mm)]}
        for kb in range(TB):
            emit_s(units[0], kb)
            emit_v_group(kb)
            for fb, ch, pool in pro_fill.get(kb, ()):
                emit_qk_group(fb, ch, pool)

        # filler op queues per unit: QK groups feed the stream two units
        # ahead; proj drains chunks closed by the preceding (1,*) unit.
        fillq = {
            0: qk_ops(3, 1) + qk_ops(3, 2) + qk_ops(3, 3) + qk_ops(0, 1),
            1: qk_ops(1, 1),
            2: qk_ops(0, 2) + sum((proj_ops(fb, 0) for fb in range(KT)), []),
            3: qk_ops(1, 2),
            4: qk_ops(0, 3) + sum((proj_ops(fb, 1) for fb in range(KT)), []),
            5: qk_ops(1, 3),
            6: (proj_ops(0, 2) + proj_ops(1, 2) + proj_ops(2, 2)
                + proj_ops(3, 2) + proj_ops(4, 2) + proj_ops(5, 2)
                + proj_ops(6, 2) + proj_ops(7, 2)),
        }
        os3 = []

        def os3_prefill(fb):
            def op():
                ps = ps_mm.tile([128, CH], f32, tag="mm", name=f"pj3a{fb}")
                nc.tensor.matmul(ps, pw_sb[0][:, fb * 128:(fb + 1) * 128],
                                 ot_sb[0][NCH - 1], start=True, stop=True)
                os = outs.tile([128, CH], f32, tag=f"os3_{fb}", bufs=1,
                               name=f"os3_{fb}")
                nc.scalar.copy(out=os, in_=ps)
                os3.append(os)
            return ("proj", op)

        fillq[7] = [os3_prefill(fb) for fb in range(KT)]

        for i, u in enumerate(units):
            hp, cc = u
            nxt = units[i + 1] if i + 1 < len(units) else None
            if nxt is not None:
                et_store[nxt] = [None] * TB
            avs = [
                ps_av.tile([65, CH], f32, tag="av", name=f"av{hp}_{cc}_{j}")
                for j in range(2)
            ]
            ops = fillq.get(i, [])
            for kb in range(TB):
                for j in range(2):
                    nc.tensor.matmul(
                        avs[j],
                        vt_sb[kb][:, 2 * hp + j, :],
                        et_store[u][kb][:, j * CH:(j + 1) * CH],
                        start=(kb == 0), stop=(kb == TB - 1),
                    )
                # pop fillers: enough each slot to drain the queue by unit end
                npop = -(-len(ops) // (TB - kb))
                for _ in range(min(npop, 3)):
                    if ops:
                        ops.pop(0)[1]()
                if nxt is not None:
                    emit_s(nxt, kb)
            for kind, op in ops:
                op()
            et_store.pop(u)

            # epilogue: drain AV psum to SBUF (frees banks), reciprocal of
            # the ones-row, broadcast via DRAM round-trip, normalize on Pool
            stg = stgp.tile([65, 2 * CH], f32, tag="stg", name=f"stg{hp}_{cc}")
            sums = recp.tile([1, 2 * CH], f32, tag="sums", name=f"sums{hp}_{cc}")
            for j in range(2):
                nc.vector.tensor_copy(out=stg[:, j * CH:(j + 1) * CH], in_=avs[j])
                nc.vector.tensor_copy(
                    out=sums[:, j * CH:(j + 1) * CH], in_=avs[j][64:65, :]
                )
            rec = recp.tile([1, 2 * CH], f32, tag="rec", name=f"rec{hp}_{cc}")
            nc.vector.reciprocal_approx_fast(out=rec, in_=sums)
            nc.gpsimd.dma_start(out=rscr.ap()[hp, cc], in_=rec)
            rec64 = recp.tile([64, 2 * CH], f32, tag="rec64", name=f"rb{hp}_{cc}")
            nc.gpsimd.dma_start(
                out=rec64, in_=rscr.ap()[hp, cc].partition_broadcast(64)
            )
            for j in range(2):
                h = 2 * hp + j
                nc.gpsimd.tensor_mul(
                    out=ot_sb[h // 2][cc][(h % 2) * 64:(h % 2) * 64 + 64, :],
                    in0=stg[0:64, j * CH:(j + 1) * CH],
                    in1=rec64[:, j * CH:(j + 1) * CH],
                )
        # chunk-3 projection tail: add the t=1 half onto the prefilled t=0
        for fb in range(KT):
            ps = ps_mm.tile([128, CH], f32, tag="mm", name=f"pj3b{fb}")
            nc.tensor.matmul(ps, pw_sb[1][:, fb * 128:(fb + 1) * 128],
                             ot_sb[1][NCH - 1], start=True, stop=True)
            nc.vector.tensor_add(out=os3[fb], in0=os3[fb], in1=ps)
            dma_rr[fb % 3].dma_start(
                out=out_r[fb][:, (NCH - 1) * CH:NCH * CH], in_=os3[fb]
            )

    nc.finalize()
    return nc


def _in_maps(x, qkv_w, qkv_b, proj_w):
    import ml_dtypes

    bf = ml_dtypes.bfloat16
    maps = []
    for c in range(NCORE):
        b, hg = c // 4, c % 4
        fs = slice(hg * F, (hg + 1) * F)
        q, k = qkv_w[fs], qkv_w[DIM:][fs]
        # device column order [k01 | q01 | k23 | q23]
        wqk = np.concatenate([k[:128], q[:128], k[128:], q[128:]], 0)  # [512,1024]
        bqk = np.concatenate([qkv_b[fs], qkv_b[DIM:][fs]], 0)
        maps.append({
            "xt": np.ascontiguousarray(x[b].T).astype(bf),
            "wqk": np.ascontiguousarray(wqk.T).astype(bf),
            "wv": np.ascontiguousarray(qkv_w[2 * DIM:][fs].T).astype(bf),
            "bqk": np.ascontiguousarray(bqk),
            "bv": np.ascontiguousarray(qkv_b[2 * DIM:][fs]),
            "pw": np.ascontiguousarray(proj_w[:, fs].T).astype(bf),
        })
    return maps


def _run(inputs, trace=False, trace_kwargs=None):
    from concourse.bass_utils import run_bass_kernel_spmd

    if "nc" not in _cache:
        _cache["nc"] = _build()
    nc = _cache["nc"]
    maps = _in_maps(inputs["x"], inputs["qkv_w"], inputs["qkv_b"], inputs["proj_w"])
    res = run_bass_kernel_spmd(
        nc, maps, list(range(NCORE)), trace=trace, **(trace_kwargs or {})
    )
    outs = [r["out"] for r in res.results]              # [1024, 2048] partials
    full = np.empty((B, N, DIM), dtype=np.float32)
    for b in range(B):
        acc = outs[4 * b].copy()
        for c in range(4 * b + 1, 4 * b + 4):
            acc += outs[c]
        full[b] = acc.T + inputs["proj_b"]
    return full, res


def kernel(**inputs) -> np.ndarray:
    out, _ = _run(inputs, trace=False)
    return out


# revision 25
# speedup vs baseline: 1.0619x; 1.0619x over previous
"""Multi-head attention (B=2, N=2048, D=1024, H=16) on 8 TRN2 NeuronCores.

Sharding: core c handles batch b=c//4 and head group hg=c%4 (4 heads of 16).
Each core computes QKV for its heads, materialized attention, and a partial
projection (proj row-split over heads); the host sums 4 partials per batch
and adds proj bias.  No device collectives.

v2 schedule, engineered to the PE roofline (~136.5us of moving-row time):
  - chunk-granular input DMA (one descriptor-batch per x chunk) spread over
    4 queues so the first score matmul fires at ~6us
  - hp-interleaved unit order (0,0),(1,0),(0,1),(1,1),... so projection
    work for chunk cc unlocks right after unit (1,cc) and spreads forward
  - exp split: most kb-slabs on ACT (hardware Exp), kbs in DVE_KBS per unit
    computed on the Vector engine with a Schraudolph fast-exp (scores*A+B
    -> int16 -> bitcast bf16), keeping ACT under the PE floor
  - V bias-add + normalize-mul + proj-psum drain on Pool, score bias +
    AV-psum drain + reciprocal on Vector: no engine above ~60% of the span
  - PSUM: 4 banks score double-buffer, 2 banks AV accumulators, 2 banks
    shared QKV/V/proj staging (prologue QK groups borrow the score banks)
"""

import numpy as np

B, N, DIM, H, DH = 2, 2048, 1024, 16, 64
SCALE = DH ** -0.5
NCORE = 8
HPC = 4            # heads per core
F = HPC * DH       # 256 features per core-headgroup
CH = 512           # token chunk (matmul moving free dim)
NCH = N // CH      # 4
KT = DIM // 128    # 8 k-tiles over model dim
TB = N // 128      # 16 token blocks
DVE_KBS = (1, 3, 6, 9, 11, 14)  # kb slabs per unit whose exp runs on DVE (fast-exp)
EXP_A = SCALE * (2.0 ** 7) / float(np.log(2.0))   # schraudolph multiplier
EXP_B = 127.0 * 128.0 - 7.0                        # schraudolph bias (c=7)
_cache = {}


def _build():
    from contextlib import ExitStack

    import concourse.mybir as mybir
    from concourse import bacc
    from concourse.tile import TileContext

    f32 = mybir.dt.float32
    bf16 = mybir.dt.bfloat16
    i16 = mybir.dt.int16
    nc = bacc.Bacc("TRN2", target_bir_lowering=False)

    xt_d = nc.declare_dram_parameter("xt", [DIM, N], bf16, isOutput=False)
    wqk_d = nc.declare_dram_parameter("wqk", [DIM, 2 * F], bf16, isOutput=False)
    wv_d = nc.declare_dram_parameter("wv", [DIM, F], bf16, isOutput=False)
    bqk_d = nc.declare_dram_parameter("bqk", [2 * F], f32, isOutput=False)
    bv_d = nc.declare_dram_parameter("bv", [F], f32, isOutput=False)
    pw_d = nc.declare_dram_parameter("pw", [F, DIM], bf16, isOutput=False)
    out_d = nc.declare_dram_parameter("out", [DIM, N], f32, isOutput=True)
    rscr = nc.dram_tensor("rscr", [2, NCH, 2 * CH], f32)

    # chunk-major views: one DMA delivers [128, 8, *] (all 8 k-tiles)
    xt_r = xt_d.ap().rearrange("(t p) n -> p t n", p=128)
    wqk_r = wqk_d.ap().rearrange("(t p) m -> p t m", p=128)
    wv_r = wv_d.ap().rearrange("(t p) m -> p t m", p=128)
    pw_r = pw_d.ap().rearrange("(t p) m -> t p m", p=128)
    out_r = out_d.ap().rearrange("(t p) n -> t p n", p=128)

    with TileContext(nc) as tc, ExitStack() as st:
        consts = st.enter_context(tc.tile_pool(name="consts", bufs=1))
        qkp = st.enter_context(tc.tile_pool(name="qkp", bufs=1))
        vtp = st.enter_context(tc.tile_pool(name="vtp", bufs=1))
        otp = st.enter_context(tc.tile_pool(name="otp", bufs=1))
        ep = st.enter_context(tc.tile_pool(name="ep", bufs=2))
        recp = st.enter_context(tc.tile_pool(name="recp", bufs=2))
        outs = st.enter_context(tc.tile_pool(name="outs", bufs=3))
        stgp = st.enter_context(tc.tile_pool(name="stgp", bufs=2))
        xw = st.enter_context(tc.tile_pool(name="xw", bufs=1))
        ps_mm = st.enter_context(tc.tile_pool(name="ps_mm", bufs=2, space="PSUM"))
        ps_s = st.enter_context(tc.tile_pool(name="ps_s", bufs=2, space="PSUM"))
        ps_av = st.enter_context(tc.tile_pool(name="ps_av", bufs=2, space="PSUM"))

        # ---- constant + weight tiles -------------------------------------
        bqk_sb = consts.tile([128, 2 * F // 128], f32)
        bv_sb = consts.tile([128, F], f32)
        # host supplies wqk with columns reordered to [k01 | q01 | k23 | q23]
        wkq01_sb = xw.tile([128, KT, F], bf16)
        wkq23_sb = xw.tile([128, KT, F], bf16)
        wv_sb = xw.tile([128, KT, F], bf16)
        pw_sb = [consts.tile([128, DIM], bf16, tag=f"pw{t}", name=f"pw{t}")
                 for t in range(2)]
        xt_sb = [xw.tile([128, KT, CH], bf16, tag=f"x{ch}", name=f"x{ch}")
                 for ch in range(NCH)]

        # DMA plan (issue ~0.6us fixed, ~150GB/s per ring, keep elements
        # >=512B).  Ring loads: sync: kq01, x0b, x1 | scalar: wv, kq23,
        # x3, pw | gpsimd: x0a, biases, x2.  k01+q01+x0 gate the first
        # score matmul.
        nc.sync.dma_start(out=wkq01_sb[:, :4, :], in_=wqk_r[:, :4, :F])
        nc.gpsimd.dma_start(out=xt_sb[0][:, :4, :], in_=xt_r[:, :4, 0 * CH:1 * CH])
        nc.scalar.dma_start(out=wv_sb, in_=wv_r)
        nc.sync.dma_start(out=wkq01_sb[:, 4:, :], in_=wqk_r[:, 4:, :F])
        nc.gpsimd.dma_start(out=bqk_sb, in_=bqk_d.ap().rearrange("(f p) -> p f", p=128))
        nc.gpsimd.dma_start(out=bv_sb, in_=bv_d.ap().partition_broadcast(128))
        nc.gpsimd.dma_start(out=xt_sb[0][:, 4:, :], in_=xt_r[:, 4:, 0 * CH:1 * CH])
        nc.scalar.dma_start(out=wkq23_sb, in_=wqk_r[:, :, F:])
        nc.sync.dma_start(out=xt_sb[1], in_=xt_r[:, :, 1 * CH:2 * CH])
        nc.gpsimd.dma_start(out=xt_sb[2], in_=xt_r[:, :, 2 * CH:3 * CH])
        nc.scalar.dma_start(out=xt_sb[3], in_=xt_r[:, :, 3 * CH:4 * CH])
        for t in range(2):
            nc.scalar.dma_start(out=pw_sb[t], in_=pw_r[t])

        # ---- working tiles ----------------------------------------------
        qk_sb = [[qkp.tile([128, CH], bf16, tag=f"qk{fb}_{ch}", name=f"qk{fb}_{ch}")
                  for ch in range(NCH)] for fb in range(4)]
        vt_sb = [vtp.tile([128, HPC, DH + 1], bf16, tag=f"vt{tb}", name=f"vt{tb}")
                 for tb in range(TB)]
        ot_sb = [[otp.tile([128, CH], bf16, tag=f"ot{t}_{ch}", name=f"ot{t}_{ch}")
                  for ch in range(NCH)] for t in range(2)]

        def emit_qk_group(fb, ch, pool):
            # fb: 0=q01 1=q23 2=k01 3=k23; host column order [k01 q01 k23 q23]
            w = wkq01_sb if fb % 2 == 0 else wkq23_sb
            wo = 0 if fb >= 2 else 128
            ps = pool.tile([128, CH], f32, tag=pool.name.startswith("ps_s") and "sp" or "mm",
                           name=f"qkg{fb}_{ch}")
            for t in range(KT):
                nc.tensor.matmul(
                    ps,
                    w[:, t, wo:wo + 128],
                    xt_sb[ch][:, t, :],
                    start=(t == 0), stop=(t == KT - 1),
                )
            nc.vector.tensor_scalar_add(
                out=qk_sb[fb][ch], in0=ps, scalar1=bqk_sb[:, fb:fb + 1],
            )

        def emit_v_group(tb):
            # prologue-only: borrows the (still unused) AV psum slots
            ps = ps_av.tile([128, F], f32, tag="av", name=f"vg{tb}")
            ch, blk = tb // 4, tb % 4
            for t in range(KT):
                nc.tensor.matmul(
                    ps,
                    xt_sb[ch][:, t, blk * 128:(blk + 1) * 128],
                    wv_sb[:, t, :],
                    start=(t == 0), stop=(t == KT - 1),
                )
            nc.vector.tensor_add(
                out=vt_sb[tb][:, :, :DH],
                in0=ps.rearrange("p (h d) -> p h d", h=HPC),
                in1=bv_sb.rearrange("p (h d) -> p h d", h=HPC),
            )
            nc.vector.memset(vt_sb[tb][:, :, DH:], 1.0)

        # ---- attention units: unit = (head-pair hp, chunk cc) ------------
        units = [(hp, cc) for cc in range(NCH) for hp in (0, 1)]
        et_store = {}

        def q_slice(h, cc):
            return qk_sb[h // 2][cc][(h % 2) * 64:(h % 2) * 64 + 64, :]

        def k_slice(h, kb):
            t = qk_sb[2 + h // 2][kb // 4]
            return t[(h % 2) * 64:(h % 2) * 64 + 64, (kb % 4) * 128:(kb % 4 + 1) * 128]

        def emit_s(u, kb):
            hp, cc = u
            sp = ps_s.tile([128, 2 * CH], f32, tag="sp", name=f"sp{hp}_{cc}_{kb}")
            for j in range(2):
                h = 2 * hp + j
                nc.tensor.matmul(
                    sp[:, j * CH:(j + 1) * CH],
                    k_slice(h, kb),
                    q_slice(h, cc),
                    start=True, stop=True,
                )
            e = ep.tile([128, 2 * CH], bf16, tag=f"e{kb}", name=f"e{hp}_{cc}_{kb}")
            if kb in DVE_KBS:
                nc.vector.tensor_scalar(
                    out=e.bitcast(i16), in0=sp,
                    scalar1=float(EXP_A), scalar2=float(EXP_B),
                    op0=mybir.AluOpType.mult, op1=mybir.AluOpType.add,
                )
            else:
                nc.scalar.activation(
                    out=e, in_=sp,
                    func=mybir.ActivationFunctionType.Exp, scale=SCALE,
                )
            et_store[u][kb] = e

        dma_rr = [nc.sync, nc.gpsimd, nc.scalar]

        def proj_ops(fb, cc):
            # projection group split into two single-matmul filler ops; the
            # psum drain runs on the (slack) ACT engine
            state = {}

            def op1():
                ps = ps_mm.tile([128, CH], f32, tag="mm", name=f"pj{fb}_{cc}")
                state["ps"] = ps
                nc.tensor.matmul(ps, pw_sb[0][:, fb * 128:(fb + 1) * 128],
                                 ot_sb[0][cc], start=True, stop=False)

            def op2():
                ps = state["ps"]
                nc.tensor.matmul(ps, pw_sb[1][:, fb * 128:(fb + 1) * 128],
                                 ot_sb[1][cc], start=False, stop=True)
                os = outs.tile([128, CH], f32, tag="os", name=f"os{fb}_{cc}")
                nc.scalar.copy(out=os, in_=ps)
                dma_rr[fb % 3].dma_start(
                    out=out_r[fb][:, cc * CH:(cc + 1) * CH], in_=os
                )

            return [("proj", op1), ("proj", op2)]

        def qk_ops(fb, ch):
            # QK group as 8 single-matmul ops + bias drain on the last
            w = wkq01_sb if fb % 2 == 0 else wkq23_sb
            wo = 0 if fb >= 2 else 128
            state = {}
            ops = []

            def mk(t):
                def op():
                    if t == 0:
                        state["ps"] = ps_mm.tile([128, CH], f32, tag="mm",
                                                 name=f"qkg{fb}_{ch}")
                    nc.tensor.matmul(state["ps"], w[:, t, wo:wo + 128],
                                     xt_sb[ch][:, t, :],
                                     start=(t == 0), stop=(t == KT - 1))
                    if t == KT - 1:
                        nc.vector.tensor_scalar_add(
                            out=qk_sb[fb][ch], in0=state["ps"],
                            scalar1=bqk_sb[:, fb:fb + 1])
                return op

            for t in range(KT):
                ops.append(("qk", mk(t)))
            return ops

        # ---- prologue: k01/q01 chunk0, then unit-0 scores + V fillers ----
        emit_qk_group(2, 0, ps_s)   # k01 c0 (borrows a score bank)
        emit_qk_group(0, 0, ps_s)   # q01 c0
        et_store[units[0]] = [None] * TB
        # per-kb filler: one V group per slab; k01(c+1) before S needs it;
        # k23/q23 chunk0 early so unit (1,0)'s score stream can start.
        pro_fill = {1: [(3, 0, ps_mm)], 3: [(2, 1, ps_mm)], 5: [(1, 0, ps_mm)],
                    7: [(2, 2, ps_mm)], 11: [(2, 3, ps_mm)]}
        for kb in range(TB):
            emit_s(units[0], kb)
            emit_v_group(kb)
            for fb, ch, pool in pro_fill.get(kb, ()):
                emit_qk_group(fb, ch, pool)

        # filler op queues per unit: QK groups feed the stream two units
        # ahead; proj drains chunks closed by the preceding (1,*) unit.
        fillq = {
            0: qk_ops(3, 1) + qk_ops(3, 2) + qk_ops(3, 3) + qk_ops(0, 1),
            1: qk_ops(1, 1),
            2: qk_ops(0, 2) + sum((proj_ops(fb, 0) for fb in range(KT)), []),
            3: qk_ops(1, 2),
            4: qk_ops(0, 3) + sum((proj_ops(fb, 1) for fb in range(KT)), []),
            5: qk_ops(1, 3),
            6: (proj_ops(0, 2) + proj_ops(1, 2) + proj_ops(2, 2)
                + proj_ops(3, 2) + proj_ops(4, 2) + proj_ops(5, 2)
                + proj_ops(6, 2) + proj_ops(7, 2)),
        }
        os3 = []

        def os3_prefill(fb):
            def op():
                ps = ps_mm.tile([128, CH], f32, tag="mm", name=f"pj3a{fb}")
                nc.tensor.matmul(ps, pw_sb[0][:, fb * 128:(fb + 1) * 128],
                                 ot_sb[0][NCH - 1], start=True, stop=True)
                os = outs.tile([128, CH], f32, tag=f"os3_{fb}", bufs=1,
                               name=f"os3_{fb}")
                nc.scalar.copy(out=os, in_=ps)
                os3.append(os)
            return ("proj", op)

        fillq[7] = [os3_prefill(fb) for fb in range(KT)]

        for i, u in enumerate(units):
            hp, cc = u
            nxt = units[i + 1] if i + 1 < len(units) else None
            if nxt is not None:
                et_store[nxt] = [None] * TB
            avs = [
                ps_av.tile([65, CH], f32, tag="av", name=f"av{hp}_{cc}_{j}")
                for j in range(2)
            ]
            ops = fillq.get(i, [])
            for kb in range(TB):
                for j in range(2):
                    nc.tensor.matmul(
                        avs[j],
                        vt_sb[kb][:, 2 * hp + j, :],
                        et_store[u][kb][:, j * CH:(j + 1) * CH],
                        start=(kb == 0), stop=(kb == TB - 1),
                    )
                # pop fillers: enough each slot to drain the queue by unit end
                npop = -(-len(ops) // (TB - kb))
                for _ in range(min(npop, 3)):
                    if ops:
                        ops.pop(0)[1]()
                if nxt is not None:
                    emit_s(nxt, kb)
            for kind, op in ops:
                op()
            et_store.pop(u)

            # epilogue: drain AV psum to SBUF (frees banks), reciprocal of
            # the ones-row, broadcast via DRAM round-trip, normalize on Pool
            stg = stgp.tile([65, 2 * CH], f32, tag="stg", name=f"stg{hp}_{cc}")
            sums = recp.tile([1, 2 * CH], f32, tag="sums", name=f"sums{hp}_{cc}")
            for j in range(2):
                nc.vector.tensor_copy(out=stg[:, j * CH:(j + 1) * CH], in_=avs[j])
                nc.vector.tensor_copy(
                    out=sums[:, j * CH:(j + 1) * CH], in_=avs[j][64:65, :]
                )
            rec = recp.tile([1, 2 * CH], f32, tag="rec", name=f"rec{hp}_{cc}")
            nc.vector.reciprocal_approx_fast(out=rec, in_=sums)
            nc.gpsimd.dma_start(out=rscr.ap()[hp, cc], in_=rec)
            rec64 = recp.tile([64, 2 * CH], f32, tag="rec64", name=f"rb{hp}_{cc}")
            nc.gpsimd.dma_start(
                out=rec64, in_=rscr.ap()[hp, cc].partition_broadcast(64)
            )
            for j in range(2):
                h = 2 * hp + j
                nc.gpsimd.tensor_mul(
                    out=ot_sb[h // 2][cc][(h % 2) * 64:(h % 2) * 64 + 64, :],
                    in0=stg[0:64, j * CH:(j + 1) * CH],
                    in1=rec64[:, j * CH:(j + 1) * CH],
                )
        # chunk-3 projection tail: add the t=1 half onto the prefilled t=0
        for fb in range(KT):
            ps = ps_mm.tile([128, CH], f32, tag="mm", name=f"pj3b{fb}")
            nc.tensor.matmul(ps, pw_sb[1][:, fb * 128:(fb + 1) * 128],
                             ot_sb[1][NCH - 1], start=True, stop=True)
            nc.vector.tensor_add(out=os3[fb], in0=os3[fb], in1=ps)
            dma_rr[fb % 3].dma_start(
                out=out_r[fb][:, (NCH - 1) * CH:NCH * CH], in_=os3[fb]
            )

    nc.finalize()
    return nc


def _in_maps(x, qkv_w, qkv_b, proj_w):
    import ml_dtypes

    bf = ml_dtypes.bfloat16
    maps = []
    for c in range(NCORE):
        b, hg = c // 4, c % 4
        fs = slice(hg * F, (hg + 1) * F)
        q, k = qkv_w[fs], qkv_w[DIM:][fs]
        # device column order [k01 | q01 | k23 | q23]
        wqk = np.concatenate([k[:128], q[:128], k[128:], q[128:]], 0)  # [512,1024]
        bqk = np.concatenate([qkv_b[fs], qkv_b[DIM:][fs]], 0)
        maps.append({
            "xt": np.ascontiguousarray(x[b].T).astype(bf),
            "wqk": np.ascontiguousarray(wqk.T).astype(bf),
            "wv": np.ascontiguousarray(qkv_w[2 * DIM:][fs].T).astype(bf),
            "bqk": np.ascontiguousarray(bqk),
            "bv": np.ascontiguousarray(qkv_b[2 * DIM:][fs]),
            "pw": np.ascontiguousarray(proj_w[:, fs].T).astype(bf),
        })
    return maps


def _run(inputs, trace=False, trace_kwargs=None):
    from concourse.bass_utils import run_bass_kernel_spmd

    if "nc" not in _cache:
        _cache["nc"] = _build()
    nc = _cache["nc"]
    maps = _in_maps(inputs["x"], inputs["qkv_w"], inputs["qkv_b"], inputs["proj_w"])
    res = run_bass_kernel_spmd(
        nc, maps, list(range(NCORE)), trace=trace, **(trace_kwargs or {})
    )
    outs = [r["out"] for r in res.results]              # [1024, 2048] partials
    full = np.empty((B, N, DIM), dtype=np.float32)
    for b in range(B):
        acc = outs[4 * b].copy()
        for c in range(4 * b + 1, 4 * b + 4):
            acc += outs[c]
        full[b] = acc.T + inputs["proj_b"]
    return full, res


def kernel(**inputs) -> np.ndarray:
    out, _ = _run(inputs, trace=False)
    return out


# revision 36
# speedup vs baseline: 1.0828x; 1.0198x over previous
"""Multi-head attention (B=2, N=2048, D=1024, H=16) on 8 TRN2 NeuronCores.

Sharding: core c handles batch b=c//4 and head group hg=c%4 (4 heads of 16).
Each core computes QKV for its heads, materialized attention, and a partial
projection (proj row-split over heads); the host sums 4 partials per batch
and adds proj bias.  No device collectives.

v2 schedule, engineered to the PE roofline (~136.5us of moving-row time):
  - chunk-granular input DMA (one descriptor-batch per x chunk) spread over
    4 queues so the first score matmul fires at ~6us
  - hp-interleaved unit order (0,0),(1,0),(0,1),(1,1),... so projection
    work for chunk cc unlocks right after unit (1,cc) and spreads forward
  - exp split: most kb-slabs on ACT (hardware Exp), kbs in DVE_KBS per unit
    computed on the Vector engine with a Schraudolph fast-exp (scores*A+B
    -> int16 -> bitcast bf16), keeping ACT under the PE floor
  - V bias-add + normalize-mul + proj-psum drain on Pool, score bias +
    AV-psum drain + reciprocal on Vector: no engine above ~60% of the span
  - PSUM: 4 banks score double-buffer, 2 banks AV accumulators, 2 banks
    shared QKV/V/proj staging (prologue QK groups borrow the score banks)
"""

import numpy as np

B, N, DIM, H, DH = 2, 2048, 1024, 16, 64
SCALE = DH ** -0.5
NCORE = 8
HPC = 4            # heads per core
F = HPC * DH       # 256 features per core-headgroup
CH = 512           # token chunk (matmul moving free dim)
NCH = N // CH      # 4
KT = DIM // 128    # 8 k-tiles over model dim
TB = N // 128      # 16 token blocks
DVE_KBS = (1, 3, 6, 9, 11, 14)  # kb slabs per unit whose exp runs on DVE (fast-exp)
K_SHIFT = 2.5      # global score shift: E = exp(s - K), cancels in softmax,
                   # keeps exp values in fp8-e4m3 range (row maxes are ~0.8-3.1)
EXP8_A = SCALE * 8.0 / float(np.log(2.0))          # schraudolph->e4m3 multiplier
EXP8_B = 7.0 * 8.0 - 0.44 - K_SHIFT * 8.0 / float(np.log(2.0))
_cache = {}


def _build():
    from contextlib import ExitStack

    import concourse.mybir as mybir
    from concourse import bacc
    from concourse.tile import TileContext

    f32 = mybir.dt.float32
    bf16 = mybir.dt.bfloat16
    f8 = mybir.dt.float8e4
    u8 = mybir.dt.uint8
    nc = bacc.Bacc("TRN2", target_bir_lowering=False)

    xt_d = nc.declare_dram_parameter("xt", [DIM, N], bf16, isOutput=False)
    wqk_d = nc.declare_dram_parameter("wqk", [DIM, 2 * F], bf16, isOutput=False)
    wv_d = nc.declare_dram_parameter("wv", [DIM, F], bf16, isOutput=False)
    bqk_d = nc.declare_dram_parameter("bqk", [2 * F], f32, isOutput=False)
    bv_d = nc.declare_dram_parameter("bv", [F], f32, isOutput=False)
    pw_d = nc.declare_dram_parameter("pw", [F, DIM], bf16, isOutput=False)
    out_d = nc.declare_dram_parameter("out", [DIM, N], f32, isOutput=True)
    rscr = nc.dram_tensor("rscr", [2, NCH, 2 * CH], f32)

    # chunk-major views: one DMA delivers [128, 8, *] (all 8 k-tiles)
    xt_r = xt_d.ap().rearrange("(t p) n -> p t n", p=128)
    wqk_r = wqk_d.ap().rearrange("(t p) m -> p t m", p=128)
    wv_r = wv_d.ap().rearrange("(t p) m -> p t m", p=128)
    pw_r = pw_d.ap().rearrange("(t p) m -> t p m", p=128)
    out_r = out_d.ap().rearrange("(t p) n -> t p n", p=128)

    with TileContext(nc) as tc, ExitStack() as st:
        consts = st.enter_context(tc.tile_pool(name="consts", bufs=1))
        qkp = st.enter_context(tc.tile_pool(name="qkp", bufs=1))
        vtp = st.enter_context(tc.tile_pool(name="vtp", bufs=1))
        otp = st.enter_context(tc.tile_pool(name="otp", bufs=1))
        ep = st.enter_context(tc.tile_pool(name="ep", bufs=2))
        recp = st.enter_context(tc.tile_pool(name="recp", bufs=2))
        outs = st.enter_context(tc.tile_pool(name="outs", bufs=3))
        stgp = st.enter_context(tc.tile_pool(name="stgp", bufs=2))
        xw = st.enter_context(tc.tile_pool(name="xw", bufs=1))
        ps_mm = st.enter_context(tc.tile_pool(name="ps_mm", bufs=2, space="PSUM"))
        ps_s = st.enter_context(tc.tile_pool(name="ps_s", bufs=2, space="PSUM"))
        ps_av = st.enter_context(tc.tile_pool(name="ps_av", bufs=2, space="PSUM"))

        # ---- constant + weight tiles -------------------------------------
        bqk_sb = consts.tile([128, 2 * F // 128], f32)
        bv_sb = consts.tile([128, F], f32)
        nks_sb = consts.tile([128, 1], f32)
        nc.vector.memset(nks_sb, -K_SHIFT)
        # host supplies wqk with columns reordered to [k01 | q01 | k23 | q23]
        wkq01_sb = xw.tile([128, KT, F], bf16)
        wkq23_sb = xw.tile([128, KT, F], bf16)
        wv_sb = xw.tile([128, KT, F], bf16)
        pw_sb = [consts.tile([128, DIM], bf16, tag=f"pw{t}", name=f"pw{t}")
                 for t in range(2)]
        xt_sb = [xw.tile([128, KT, CH], bf16, tag=f"x{ch}", name=f"x{ch}")
                 for ch in range(NCH)]

        # DMA plan (issue ~0.6us fixed, ~150GB/s per ring, keep elements
        # >=512B).  Ring loads: sync: kq01, x0b, x1 | scalar: wv, kq23,
        # x3, pw | gpsimd: x0a, biases, x2.  k01+q01+x0 gate the first
        # score matmul.
        nc.sync.dma_start(out=wkq01_sb[:, :4, :], in_=wqk_r[:, :4, :F])
        nc.gpsimd.dma_start(out=xt_sb[0][:, :4, :], in_=xt_r[:, :4, 0 * CH:1 * CH])
        nc.scalar.dma_start(out=wv_sb, in_=wv_r)
        nc.sync.dma_start(out=wkq01_sb[:, 4:, :], in_=wqk_r[:, 4:, :F])
        nc.gpsimd.dma_start(out=bqk_sb, in_=bqk_d.ap().rearrange("(f p) -> p f", p=128))
        nc.gpsimd.dma_start(out=bv_sb, in_=bv_d.ap().partition_broadcast(128))
        nc.gpsimd.dma_start(out=xt_sb[0][:, 4:, :], in_=xt_r[:, 4:, 0 * CH:1 * CH])
        nc.scalar.dma_start(out=wkq23_sb, in_=wqk_r[:, :, F:])
        nc.sync.dma_start(out=xt_sb[1], in_=xt_r[:, :, 1 * CH:2 * CH])
        nc.gpsimd.dma_start(out=xt_sb[2], in_=xt_r[:, :, 2 * CH:3 * CH])
        nc.scalar.dma_start(out=xt_sb[3], in_=xt_r[:, :, 3 * CH:4 * CH])
        for t in range(2):
            nc.scalar.dma_start(out=pw_sb[t], in_=pw_r[t])

        # ---- working tiles ----------------------------------------------
        qk_sb = [[qkp.tile([128, CH], bf16, tag=f"qk{fb}_{ch}", name=f"qk{fb}_{ch}")
                  for ch in range(NCH)] for fb in range(4)]
        # fp8 V, two token-blocks per tile (the DoubleRow k-subtile pair)
        vt_sb = [vtp.tile([128, 2, HPC, DH + 4], f8, tag=f"vt{tb2}", name=f"vt{tb2}")
                 for tb2 in range(TB // 2)]
        ot_sb = [[otp.tile([128, CH], bf16, tag=f"ot{t}_{ch}", name=f"ot{t}_{ch}")
                  for ch in range(NCH)] for t in range(2)]

        def emit_qk_group(fb, ch, pool):
            # fb: 0=q01 1=q23 2=k01 3=k23; host column order [k01 q01 k23 q23]
            w = wkq01_sb if fb % 2 == 0 else wkq23_sb
            wo = 0 if fb >= 2 else 128
            ps = pool.tile([128, CH], f32, tag=pool.name.startswith("ps_s") and "sp" or "mm",
                           name=f"qkg{fb}_{ch}")
            for t in range(KT):
                nc.tensor.matmul(
                    ps,
                    w[:, t, wo:wo + 128],
                    xt_sb[ch][:, t, :],
                    start=(t == 0), stop=(t == KT - 1),
                )
            nc.vector.tensor_scalar_add(
                out=qk_sb[fb][ch], in0=ps, scalar1=bqk_sb[:, fb:fb + 1],
            )

        def emit_v_group(tb):
            # prologue-only: borrows the (still unused) AV psum slots
            ps = ps_av.tile([128, F], f32, tag="av", name=f"vg{tb}")
            ch, blk = tb // 4, tb % 4
            for t in range(KT):
                nc.tensor.matmul(
                    ps,
                    xt_sb[ch][:, t, blk * 128:(blk + 1) * 128],
                    wv_sb[:, t, :],
                    start=(t == 0), stop=(t == KT - 1),
                )
            nc.vector.tensor_add(
                out=vt_sb[tb // 2][:, tb % 2, :, :DH],
                in0=ps.rearrange("p (h d) -> p h d", h=HPC),
                in1=bv_sb.rearrange("p (h d) -> p h d", h=HPC),
            )
            nc.vector.memset(vt_sb[tb // 2][:, tb % 2, :, DH:], 0.0)
            nc.vector.memset(vt_sb[tb // 2][:, tb % 2, :, DH:DH + 1], 1.0)

        # ---- attention units: unit = (head-pair hp, chunk cc) ------------
        units = [(hp, cc) for cc in range(NCH) for hp in (0, 1)]
        et_store = {}

        def q_slice(h, cc):
            return qk_sb[h // 2][cc][(h % 2) * 64:(h % 2) * 64 + 64, :]

        def k_slice(h, kb):
            t = qk_sb[2 + h // 2][kb // 4]
            return t[(h % 2) * 64:(h % 2) * 64 + 64, (kb % 4) * 128:(kb % 4 + 1) * 128]

        def emit_s(u, kb):
            # scores for slab kb; exp -> fp8 half of the paired E tile
            hp, cc = u
            sp = ps_s.tile([128, 2 * CH], f32, tag="sp", name=f"sp{hp}_{cc}_{kb}")
            for j in range(2):
                h = 2 * hp + j
                nc.tensor.matmul(
                    sp[:, j * CH:(j + 1) * CH],
                    k_slice(h, kb),
                    q_slice(h, cc),
                    start=True, stop=True,
                )
            kb2 = kb // 2
            if kb % 2 == 0:
                e = ep.tile([128, 2, 2 * CH], f8, tag=f"e{kb2}",
                            name=f"e{hp}_{cc}_{kb2}")
                et_store[u][kb2] = e
            else:
                e = et_store[u][kb2]
            eh = e[:, kb % 2, :]
            if kb in DVE_KBS:
                nc.vector.tensor_scalar(
                    out=eh.bitcast(u8), in0=sp,
                    scalar1=float(EXP8_A), scalar2=float(EXP8_B),
                    op0=mybir.AluOpType.mult, op1=mybir.AluOpType.add,
                )
            else:
                nc.scalar.activation(
                    out=eh, in_=sp,
                    func=mybir.ActivationFunctionType.Exp, scale=SCALE,
                    bias=nks_sb[:, 0:1],
                )
            et_store[u][kb2] = e

        dma_rr = [nc.sync, nc.gpsimd, nc.scalar]

        def proj_ops(fb, cc):
            # projection group split into two single-matmul filler ops; the
            # psum drain runs on the (slack) ACT engine
            state = {}

            def op1():
                ps = ps_mm.tile([128, CH], f32, tag="mm", name=f"pj{fb}_{cc}")
                state["ps"] = ps
                nc.tensor.matmul(ps, pw_sb[0][:, fb * 128:(fb + 1) * 128],
                                 ot_sb[0][cc], start=True, stop=False)

            def op2():
                ps = state["ps"]
                nc.tensor.matmul(ps, pw_sb[1][:, fb * 128:(fb + 1) * 128],
                                 ot_sb[1][cc], start=False, stop=True)
                os = outs.tile([128, CH], f32, tag="os", name=f"os{fb}_{cc}")
                nc.scalar.copy(out=os, in_=ps)
                dma_rr[fb % 3].dma_start(
                    out=out_r[fb][:, cc * CH:(cc + 1) * CH], in_=os
                )

            return [("proj", op1), ("proj", op2)]

        def qk_ops(fb, ch):
            # QK group as 8 single-matmul ops + bias drain on the last
            w = wkq01_sb if fb % 2 == 0 else wkq23_sb
            wo = 0 if fb >= 2 else 128
            state = {}
            ops = []

            def mk(t):
                def op():
                    if t == 0:
                        state["ps"] = ps_mm.tile([128, CH], f32, tag="mm",
                                                 name=f"qkg{fb}_{ch}")
                    nc.tensor.matmul(state["ps"], w[:, t, wo:wo + 128],
                                     xt_sb[ch][:, t, :],
                                     start=(t == 0), stop=(t == KT - 1))
                    if t == KT - 1:
                        nc.vector.tensor_scalar_add(
                            out=qk_sb[fb][ch], in0=state["ps"],
                            scalar1=bqk_sb[:, fb:fb + 1])
                return op

            for t in range(KT):
                ops.append(("qk", mk(t)))
            return ops

        # ---- prologue: k01/q01 chunk0, then unit-0 scores + V fillers ----
        emit_qk_group(2, 0, ps_s)   # k01 c0 (borrows a score bank)
        emit_qk_group(0, 0, ps_s)   # q01 c0
        et_store[units[0]] = [None] * (TB // 2)
        # per-kb filler: one V group per slab; k01(c+1) before S needs it;
        # k23/q23 chunk0 early so unit (1,0)'s score stream can start.
        pro_fill = {1: [(3, 0, ps_mm)], 3: [(2, 1, ps_mm)], 5: [(1, 0, ps_mm)],
                    7: [(2, 2, ps_mm)], 11: [(2, 3, ps_mm)]}
        for kb in range(TB):
            emit_s(units[0], kb)
            emit_v_group(kb)
            for fb, ch, pool in pro_fill.get(kb, ()):
                emit_qk_group(fb, ch, pool)

        # filler op queues per unit: QK groups feed the stream two units
        # ahead; proj drains chunks closed by the preceding (1,*) unit.
        fillq = {
            0: qk_ops(3, 1) + qk_ops(3, 2) + qk_ops(3, 3) + qk_ops(0, 1),
            1: qk_ops(1, 1),
            2: qk_ops(0, 2) + sum((proj_ops(fb, 0) for fb in range(KT)), []),
            3: qk_ops(1, 2),
            4: qk_ops(0, 3) + sum((proj_ops(fb, 1) for fb in range(KT)), []),
            5: qk_ops(1, 3),
            6: (proj_ops(0, 2) + proj_ops(1, 2) + proj_ops(2, 2)
                + proj_ops(3, 2) + proj_ops(4, 2) + proj_ops(5, 2)
                + proj_ops(6, 2) + proj_ops(7, 2)),
        }
        os3 = []

        def os3_prefill(fb):
            def op():
                ps = ps_mm.tile([128, CH], f32, tag="mm", name=f"pj3a{fb}")
                nc.tensor.matmul(ps, pw_sb[0][:, fb * 128:(fb + 1) * 128],
                                 ot_sb[0][NCH - 1], start=True, stop=True)
                os = outs.tile([128, CH], f32, tag=f"os3_{fb}", bufs=1,
                               name=f"os3_{fb}")
                nc.scalar.copy(out=os, in_=ps)
                os3.append(os)
            return ("proj", op)

        fillq[7] = [os3_prefill(fb) for fb in range(KT)]

        for i, u in enumerate(units):
            hp, cc = u
            nxt = units[i + 1] if i + 1 < len(units) else None
            if nxt is not None:
                et_store[nxt] = [None] * (TB // 2)
            avs = [
                ps_av.tile([68, CH], f32, tag="av", name=f"av{hp}_{cc}_{j}")
                for j in range(2)
            ]
            ops = fillq.get(i, [])
            for kb in range(TB):
                # one fp8 DoubleRow AV matmul per slot (2 k-subtiles each):
                # head j = kb%2 over token-block pair kb2 = kb//2
                j, kb2 = kb % 2, kb // 2
                nc.tensor.matmul(
                    avs[j],
                    vt_sb[kb2][:, :, 2 * hp + j, :],
                    et_store[u][kb2][:, :, j * CH:(j + 1) * CH],
                    start=(kb2 == 0), stop=(kb2 == TB // 2 - 1),
                    perf_mode=mybir.MatmulPerfMode.DoubleRow,
                )
                # pop fillers: enough each slot to drain the queue by unit end
                npop = -(-len(ops) // (TB - kb))
                for _ in range(min(npop, 3)):
                    if ops:
                        ops.pop(0)[1]()
                if nxt is not None:
                    emit_s(nxt, kb)
            for kind, op in ops:
                op()
            et_store.pop(u)

            # epilogue: drain AV psum to SBUF (frees banks), reciprocal of
            # the ones-row, broadcast via DRAM round-trip, normalize on Pool
            stg = stgp.tile([65, 2 * CH], f32, tag="stg", name=f"stg{hp}_{cc}")
            sums = recp.tile([1, 2 * CH], f32, tag="sums", name=f"sums{hp}_{cc}")
            for j in range(2):
                nc.vector.tensor_copy(out=stg[:, j * CH:(j + 1) * CH], in_=avs[j][:65, :])
                nc.vector.tensor_copy(
                    out=sums[:, j * CH:(j + 1) * CH], in_=avs[j][64:65, :]
                )
            rec = recp.tile([1, 2 * CH], f32, tag="rec", name=f"rec{hp}_{cc}")
            nc.vector.reciprocal_approx_fast(out=rec, in_=sums)
            nc.gpsimd.dma_start(out=rscr.ap()[hp, cc], in_=rec)
            rec64 = recp.tile([64, 2 * CH], f32, tag="rec64", name=f"rb{hp}_{cc}")
            nc.gpsimd.dma_start(
                out=rec64, in_=rscr.ap()[hp, cc].partition_broadcast(64)
            )
            for j in range(2):
                h = 2 * hp + j
                nc.gpsimd.tensor_mul(
                    out=ot_sb[h // 2][cc][(h % 2) * 64:(h % 2) * 64 + 64, :],
                    in0=stg[0:64, j * CH:(j + 1) * CH],
                    in1=rec64[:, j * CH:(j + 1) * CH],
                )
        # chunk-3 projection tail: add the t=1 half onto the prefilled t=0
        for fb in range(KT):
            ps = ps_mm.tile([128, CH], f32, tag="mm", name=f"pj3b{fb}")
            nc.tensor.matmul(ps, pw_sb[1][:, fb * 128:(fb + 1) * 128],
                             ot_sb[1][NCH - 1], start=True, stop=True)
            nc.vector.tensor_add(out=os3[fb], in0=os3[fb], in1=ps)
            dma_rr[fb % 3].dma_start(
                out=out_r[fb][:, (NCH - 1) * CH:NCH * CH], in_=os3[fb]
            )

    nc.finalize()
    return nc


def _in_maps(x, qkv_w, qkv_b, proj_w):
    import ml_dtypes

    bf = ml_dtypes.bfloat16
    maps = []
    for c in range(NCORE):
        b, hg = c // 4, c % 4
        fs = slice(hg * F, (hg + 1) * F)
        q, k = qkv_w[fs], qkv_w[DIM:][fs]
        # device column order [k01 | q01 | k23 | q23]
        wqk = np.concatenate([k[:128], q[:128], k[128:], q[128:]], 0)  # [512,1024]
        bqk = np.concatenate([qkv_b[fs], qkv_b[DIM:][fs]], 0)
        maps.append({
            "xt": np.ascontiguousarray(x[b].T).astype(bf),
            "wqk": np.ascontiguousarray(wqk.T).astype(bf),
            "wv": np.ascontiguousarray(qkv_w[2 * DIM:][fs].T).astype(bf),
            "bqk": np.ascontiguousarray(bqk),
            "bv": np.ascontiguousarray(qkv_b[2 * DIM:][fs]),
            "pw": np.ascontiguousarray(proj_w[:, fs].T).astype(bf),
        })
    return maps


def _run(inputs, trace=False, trace_kwargs=None):
    from concourse.bass_utils import run_bass_kernel_spmd

    if "nc" not in _cache:
        _cache["nc"] = _build()
    nc = _cache["nc"]
    maps = _in_maps(inputs["x"], inputs["qkv_w"], inputs["qkv_b"], inputs["proj_w"])
    res = run_bass_kernel_spmd(
        nc, maps, list(range(NCORE)), trace=trace, **(trace_kwargs or {})
    )
    outs = [r["out"] for r in res.results]              # [1024, 2048] partials
    full = np.empty((B, N, DIM), dtype=np.float32)
    for b in range(B):
        acc = outs[4 * b].copy()
        for c in range(4 * b + 1, 4 * b + 4):
            acc += outs[c]
        full[b] = acc.T + inputs["proj_b"]
    return full, res


def kernel(**inputs) -> np.ndarray:
    out, _ = _run(inputs, trace=False)
    return out


# revision 37
# speedup vs baseline: 1.1233x; 1.0373x over previous
"""Multi-head attention (B=2, N=2048, D=1024, H=16) on 8 TRN2 NeuronCores.

Sharding: core c handles batch b=c//4 and head group hg=c%4 (4 heads of 16).
Each core computes QKV for its heads, materialized attention, and a partial
projection (proj row-split over heads); the host sums 4 partials per batch
and adds proj bias.  No device collectives.

v2 schedule, engineered to the PE roofline (~136.5us of moving-row time):
  - chunk-granular input DMA (one descriptor-batch per x chunk) spread over
    4 queues so the first score matmul fires at ~6us
  - hp-interleaved unit order (0,0),(1,0),(0,1),(1,1),... so projection
    work for chunk cc unlocks right after unit (1,cc) and spreads forward
  - exp split: most kb-slabs on ACT (hardware Exp), kbs in DVE_KBS per unit
    computed on the Vector engine with a Schraudolph fast-exp (scores*A+B
    -> int16 -> bitcast bf16), keeping ACT under the PE floor
  - V bias-add + normalize-mul + proj-psum drain on Pool, score bias +
    AV-psum drain + reciprocal on Vector: no engine above ~60% of the span
  - PSUM: 4 banks score double-buffer, 2 banks AV accumulators, 2 banks
    shared QKV/V/proj staging (prologue QK groups borrow the score banks)
"""

import numpy as np

B, N, DIM, H, DH = 2, 2048, 1024, 16, 64
SCALE = DH ** -0.5
NCORE = 8
HPC = 4            # heads per core
F = HPC * DH       # 256 features per core-headgroup
CH = 512           # token chunk (matmul moving free dim)
NCH = N // CH      # 4
KT = DIM // 128    # 8 k-tiles over model dim
TB = N // 128      # 16 token blocks
DVE_KBS = (2, 6, 10, 14)  # kb slabs per unit whose exp runs on DVE (fast-exp)
K_SHIFT = 2.5      # global score shift: E = exp(s - K), cancels in softmax,
                   # keeps exp values in fp8-e4m3 range (row maxes are ~0.8-3.1)
EXP8_A = SCALE * 8.0 / float(np.log(2.0))          # schraudolph->e4m3 multiplier
EXP8_B = 7.0 * 8.0 - 0.44 - K_SHIFT * 8.0 / float(np.log(2.0))
_cache = {}


def _build():
    from contextlib import ExitStack

    import concourse.mybir as mybir
    from concourse import bacc
    from concourse.tile import TileContext

    f32 = mybir.dt.float32
    bf16 = mybir.dt.bfloat16
    f8 = mybir.dt.float8e4
    u8 = mybir.dt.uint8
    nc = bacc.Bacc("TRN2", target_bir_lowering=False)

    xt_d = nc.declare_dram_parameter("xt", [DIM, N], bf16, isOutput=False)
    wqk_d = nc.declare_dram_parameter("wqk", [DIM, 2 * F], bf16, isOutput=False)
    wv_d = nc.declare_dram_parameter("wv", [DIM, F], bf16, isOutput=False)
    bqk_d = nc.declare_dram_parameter("bqk", [2 * F], f32, isOutput=False)
    bv_d = nc.declare_dram_parameter("bv", [F], f32, isOutput=False)
    pw_d = nc.declare_dram_parameter("pw", [F, DIM], bf16, isOutput=False)
    out_d = nc.declare_dram_parameter("out", [DIM, N], f32, isOutput=True)
    rscr = nc.dram_tensor("rscr", [2, NCH, 2 * CH], f32)

    # chunk-major views: one DMA delivers [128, 8, *] (all 8 k-tiles)
    xt_r = xt_d.ap().rearrange("(t p) n -> p t n", p=128)
    wqk_r = wqk_d.ap().rearrange("(t p) m -> p t m", p=128)
    wv_r = wv_d.ap().rearrange("(t p) m -> p t m", p=128)
    pw_r = pw_d.ap().rearrange("(t p) m -> t p m", p=128)
    out_r = out_d.ap().rearrange("(t p) n -> t p n", p=128)

    with TileContext(nc) as tc, ExitStack() as st:
        consts = st.enter_context(tc.tile_pool(name="consts", bufs=1))
        qkp = st.enter_context(tc.tile_pool(name="qkp", bufs=1))
        vtp = st.enter_context(tc.tile_pool(name="vtp", bufs=1))
        otp = st.enter_context(tc.tile_pool(name="otp", bufs=1))
        ep = st.enter_context(tc.tile_pool(name="ep", bufs=2))
        recp = st.enter_context(tc.tile_pool(name="recp", bufs=2))
        outs = st.enter_context(tc.tile_pool(name="outs", bufs=3))
        stgp = st.enter_context(tc.tile_pool(name="stgp", bufs=2))
        xw = st.enter_context(tc.tile_pool(name="xw", bufs=1))
        ps_mm = st.enter_context(tc.tile_pool(name="ps_mm", bufs=2, space="PSUM"))
        ps_s = st.enter_context(tc.tile_pool(name="ps_s", bufs=2, space="PSUM"))
        ps_av = st.enter_context(tc.tile_pool(name="ps_av", bufs=2, space="PSUM"))

        # ---- constant + weight tiles -------------------------------------
        bqk_sb = consts.tile([128, 2 * F // 128], f32)
        bv_sb = consts.tile([128, F], f32)
        nks_sb = consts.tile([128, 1], f32)
        nc.vector.memset(nks_sb, -K_SHIFT)
        # host supplies wqk with columns reordered to [k01 | q01 | k23 | q23]
        wkq01_sb = xw.tile([128, KT, F], bf16)
        wkq23_sb = xw.tile([128, KT, F], bf16)
        wv_sb = xw.tile([128, KT, F], bf16)
        pw_sb = [consts.tile([128, DIM], bf16, tag=f"pw{t}", name=f"pw{t}")
                 for t in range(2)]
        xt_sb = [xw.tile([128, KT, CH], bf16, tag=f"x{ch}", name=f"x{ch}")
                 for ch in range(NCH)]

        # DMA plan (issue ~0.6us fixed, ~150GB/s per ring, keep elements
        # >=512B).  Ring loads: sync: kq01, x0b, x1 | scalar: wv, kq23,
        # x3, pw | gpsimd: x0a, biases, x2.  k01+q01+x0 gate the first
        # score matmul.
        nc.sync.dma_start(out=wkq01_sb[:, :4, :], in_=wqk_r[:, :4, :F])
        nc.gpsimd.dma_start(out=xt_sb[0][:, :4, :], in_=xt_r[:, :4, 0 * CH:1 * CH])
        nc.scalar.dma_start(out=wv_sb, in_=wv_r)
        nc.sync.dma_start(out=wkq01_sb[:, 4:, :], in_=wqk_r[:, 4:, :F])
        nc.gpsimd.dma_start(out=bqk_sb, in_=bqk_d.ap().rearrange("(f p) -> p f", p=128))
        nc.gpsimd.dma_start(out=bv_sb, in_=bv_d.ap().partition_broadcast(128))
        nc.gpsimd.dma_start(out=xt_sb[0][:, 4:, :], in_=xt_r[:, 4:, 0 * CH:1 * CH])
        nc.scalar.dma_start(out=wkq23_sb, in_=wqk_r[:, :, F:])
        nc.sync.dma_start(out=xt_sb[1], in_=xt_r[:, :, 1 * CH:2 * CH])
        nc.gpsimd.dma_start(out=xt_sb[2], in_=xt_r[:, :, 2 * CH:3 * CH])
        nc.scalar.dma_start(out=xt_sb[3], in_=xt_r[:, :, 3 * CH:4 * CH])
        for t in range(2):
            nc.scalar.dma_start(out=pw_sb[t], in_=pw_r[t])

        # ---- working tiles ----------------------------------------------
        qk_sb = [[qkp.tile([128, CH], bf16, tag=f"qk{fb}_{ch}", name=f"qk{fb}_{ch}")
                  for ch in range(NCH)] for fb in range(4)]
        # fp8 V, two token-blocks per tile (the DoubleRow k-subtile pair)
        vt_sb = [vtp.tile([128, 2, HPC, DH + 4], f8, tag=f"vt{tb2}", name=f"vt{tb2}")
                 for tb2 in range(TB // 2)]
        ot_sb = [[otp.tile([128, CH], bf16, tag=f"ot{t}_{ch}", name=f"ot{t}_{ch}")
                  for ch in range(NCH)] for t in range(2)]

        def emit_qk_group(fb, ch, pool):
            # fb: 0=q01 1=q23 2=k01 3=k23; host column order [k01 q01 k23 q23]
            w = wkq01_sb if fb % 2 == 0 else wkq23_sb
            wo = 0 if fb >= 2 else 128
            ps = pool.tile([128, CH], f32, tag=pool.name.startswith("ps_s") and "sp" or "mm",
                           name=f"qkg{fb}_{ch}")
            for t in range(KT):
                nc.tensor.matmul(
                    ps,
                    w[:, t, wo:wo + 128],
                    xt_sb[ch][:, t, :],
                    start=(t == 0), stop=(t == KT - 1),
                )
            nc.scalar.activation(
                out=qk_sb[fb][ch], in_=ps,
                func=mybir.ActivationFunctionType.Identity,
                bias=bqk_sb[:, fb:fb + 1],
            )

        def emit_v_group(tb):
            # prologue-only: borrows the (still unused) AV psum slots
            ps = ps_av.tile([128, F], f32, tag="av", name=f"vg{tb}")
            ch, blk = tb // 4, tb % 4
            for t in range(KT):
                nc.tensor.matmul(
                    ps,
                    xt_sb[ch][:, t, blk * 128:(blk + 1) * 128],
                    wv_sb[:, t, :],
                    start=(t == 0), stop=(t == KT - 1),
                )
            nc.vector.tensor_add(
                out=vt_sb[tb // 2][:, tb % 2, :, :DH],
                in0=ps.rearrange("p (h d) -> p h d", h=HPC),
                in1=bv_sb.rearrange("p (h d) -> p h d", h=HPC),
            )
            nc.vector.memset(vt_sb[tb // 2][:, tb % 2, :, DH:], 0.0)
            nc.vector.memset(vt_sb[tb // 2][:, tb % 2, :, DH:DH + 1], 1.0)

        # ---- attention units: unit = (head-pair hp, chunk cc) ------------
        units = [(hp, cc) for cc in range(NCH) for hp in (0, 1)]
        et_store = {}

        def q_slice(h, cc):
            return qk_sb[h // 2][cc][(h % 2) * 64:(h % 2) * 64 + 64, :]

        def k_slice(h, kb):
            t = qk_sb[2 + h // 2][kb // 4]
            return t[(h % 2) * 64:(h % 2) * 64 + 64, (kb % 4) * 128:(kb % 4 + 1) * 128]

        def emit_s(u, kb):
            # scores for slab kb; exp -> fp8 half of the paired E tile
            hp, cc = u
            sp = ps_s.tile([128, 2 * CH], f32, tag="sp", name=f"sp{hp}_{cc}_{kb}")
            for j in range(2):
                h = 2 * hp + j
                nc.tensor.matmul(
                    sp[:, j * CH:(j + 1) * CH],
                    k_slice(h, kb),
                    q_slice(h, cc),
                    start=True, stop=True,
                )
            kb2 = kb // 2
            if kb % 2 == 0:
                e = ep.tile([128, 2, 2 * CH], f8, tag=f"e{kb2}",
                            name=f"e{hp}_{cc}_{kb2}")
                et_store[u][kb2] = e
            else:
                e = et_store[u][kb2]
            eh = e[:, kb % 2, :]
            if kb in DVE_KBS:
                nc.vector.tensor_scalar(
                    out=eh.bitcast(u8), in0=sp,
                    scalar1=float(EXP8_A), scalar2=float(EXP8_B),
                    op0=mybir.AluOpType.mult, op1=mybir.AluOpType.add,
                )
            else:
                nc.scalar.activation(
                    out=eh, in_=sp,
                    func=mybir.ActivationFunctionType.Exp, scale=SCALE,
                    bias=nks_sb[:, 0:1],
                )
            et_store[u][kb2] = e

        dma_rr = [nc.sync, nc.gpsimd, nc.scalar]

        def proj_ops(fb, cc):
            # projection group split into two single-matmul filler ops; the
            # psum drain runs on the (slack) ACT engine
            state = {}

            def op1():
                ps = ps_mm.tile([128, CH], f32, tag="mm", name=f"pj{fb}_{cc}")
                state["ps"] = ps
                nc.tensor.matmul(ps, pw_sb[0][:, fb * 128:(fb + 1) * 128],
                                 ot_sb[0][cc], start=True, stop=False)

            def op2():
                ps = state["ps"]
                nc.tensor.matmul(ps, pw_sb[1][:, fb * 128:(fb + 1) * 128],
                                 ot_sb[1][cc], start=False, stop=True)
                os = outs.tile([128, CH], f32, tag="os", name=f"os{fb}_{cc}")
                nc.vector.tensor_copy(out=os, in_=ps)
                dma_rr[fb % 3].dma_start(
                    out=out_r[fb][:, cc * CH:(cc + 1) * CH], in_=os
                )

            return [("proj", op1), ("proj", op2)]

        def qk_ops(fb, ch):
            # QK group as 8 single-matmul ops + bias drain on the last
            w = wkq01_sb if fb % 2 == 0 else wkq23_sb
            wo = 0 if fb >= 2 else 128
            state = {}
            ops = []

            def mk(t):
                def op():
                    if t == 0:
                        state["ps"] = ps_mm.tile([128, CH], f32, tag="mm",
                                                 name=f"qkg{fb}_{ch}")
                    nc.tensor.matmul(state["ps"], w[:, t, wo:wo + 128],
                                     xt_sb[ch][:, t, :],
                                     start=(t == 0), stop=(t == KT - 1))
                    if t == KT - 1:
                        nc.scalar.activation(
                            out=qk_sb[fb][ch], in_=state["ps"],
                            func=mybir.ActivationFunctionType.Identity,
                            bias=bqk_sb[:, fb:fb + 1])
                return op

            for t in range(KT):
                ops.append(("qk", mk(t)))
            return ops

        # ---- prologue: k01/q01 chunk0, then unit-0 scores + V fillers ----
        emit_qk_group(2, 0, ps_s)   # k01 c0 (borrows a score bank)
        emit_qk_group(0, 0, ps_s)   # q01 c0
        et_store[units[0]] = [None] * (TB // 2)
        # per-kb filler: one V group per slab; k01(c+1) before S needs it;
        # k23/q23 chunk0 early so unit (1,0)'s score stream can start.
        pro_fill = {1: [(3, 0, ps_mm)], 3: [(2, 1, ps_mm)], 5: [(1, 0, ps_mm)],
                    7: [(2, 2, ps_mm)], 11: [(2, 3, ps_mm)]}
        for kb in range(TB):
            emit_s(units[0], kb)
            emit_v_group(kb)
            for fb, ch, pool in pro_fill.get(kb, ()):
                emit_qk_group(fb, ch, pool)

        # filler op queues per unit: QK groups feed the stream two units
        # ahead; proj drains chunks closed by the preceding (1,*) unit.
        fillq = {
            0: qk_ops(3, 1) + qk_ops(3, 2) + qk_ops(3, 3) + qk_ops(0, 1),
            1: qk_ops(1, 1),
            2: qk_ops(0, 2) + sum((proj_ops(fb, 0) for fb in range(KT)), []),
            3: qk_ops(1, 2),
            4: qk_ops(0, 3) + sum((proj_ops(fb, 1) for fb in range(KT)), []),
            5: qk_ops(1, 3),
            6: (proj_ops(0, 2) + proj_ops(1, 2) + proj_ops(2, 2)
                + proj_ops(3, 2) + proj_ops(4, 2) + proj_ops(5, 2)
                + proj_ops(6, 2) + proj_ops(7, 2)),
        }
        os3 = []

        def os3_prefill(fb):
            def op():
                ps = ps_mm.tile([128, CH], f32, tag="mm", name=f"pj3a{fb}")
                nc.tensor.matmul(ps, pw_sb[0][:, fb * 128:(fb + 1) * 128],
                                 ot_sb[0][NCH - 1], start=True, stop=True)
                os = outs.tile([128, CH], f32, tag=f"os3_{fb}", bufs=1,
                               name=f"os3_{fb}")
                nc.vector.tensor_copy(out=os, in_=ps)
                os3.append(os)
            return ("proj", op)

        fillq[7] = [os3_prefill(fb) for fb in range(KT)]

        for i, u in enumerate(units):
            hp, cc = u
            nxt = units[i + 1] if i + 1 < len(units) else None
            if nxt is not None:
                et_store[nxt] = [None] * (TB // 2)
            avs = [
                ps_av.tile([68, CH], f32, tag="av", name=f"av{hp}_{cc}_{j}")
                for j in range(2)
            ]
            ops = fillq.get(i, [])
            for kb in range(TB):
                if nxt is not None:
                    emit_s(nxt, kb)
                # one fp8 DoubleRow AV matmul per slot (2 k-subtiles each):
                # head j = kb%2 over token-block pair kb2 = kb//2
                j, kb2 = kb % 2, kb // 2
                nc.tensor.matmul(
                    avs[j],
                    vt_sb[kb2][:, :, 2 * hp + j, :],
                    et_store[u][kb2][:, :, j * CH:(j + 1) * CH],
                    start=(kb2 == 0), stop=(kb2 == TB // 2 - 1),
                    perf_mode=mybir.MatmulPerfMode.DoubleRow,
                )
                # pop fillers: enough each slot to drain the queue by unit end
                npop = -(-len(ops) // (TB - kb))
                for _ in range(min(npop, 3)):
                    if ops:
                        ops.pop(0)[1]()
            for kind, op in ops:
                op()
            et_store.pop(u)

            # epilogue: drain AV psum to SBUF (frees banks), reciprocal of
            # the ones-row, broadcast via DRAM round-trip, normalize on Pool
            stg = stgp.tile([65, 2 * CH], f32, tag="stg", name=f"stg{hp}_{cc}")
            sums = recp.tile([1, 2 * CH], f32, tag="sums", name=f"sums{hp}_{cc}")
            for j in range(2):
                nc.vector.tensor_copy(out=stg[:, j * CH:(j + 1) * CH], in_=avs[j][:65, :])
                nc.vector.tensor_copy(
                    out=sums[:, j * CH:(j + 1) * CH], in_=avs[j][64:65, :]
                )
            rec = recp.tile([1, 2 * CH], f32, tag="rec", name=f"rec{hp}_{cc}")
            nc.vector.reciprocal_approx_fast(out=rec, in_=sums)
            nc.gpsimd.dma_start(out=rscr.ap()[hp, cc], in_=rec)
            rec64 = recp.tile([64, 2 * CH], f32, tag="rec64", name=f"rb{hp}_{cc}")
            nc.gpsimd.dma_start(
                out=rec64, in_=rscr.ap()[hp, cc].partition_broadcast(64)
            )
            for j in range(2):
                h = 2 * hp + j
                nc.gpsimd.tensor_mul(
                    out=ot_sb[h // 2][cc][(h % 2) * 64:(h % 2) * 64 + 64, :],
                    in0=stg[0:64, j * CH:(j + 1) * CH],
                    in1=rec64[:, j * CH:(j + 1) * CH],
                )
        # chunk-3 projection tail: add the t=1 half onto the prefilled t=0
        for fb in range(KT):
            ps = ps_mm.tile([128, CH], f32, tag="mm", name=f"pj3b{fb}")
            nc.tensor.matmul(ps, pw_sb[1][:, fb * 128:(fb + 1) * 128],
                             ot_sb[1][NCH - 1], start=True, stop=True)
            nc.vector.tensor_add(out=os3[fb], in0=os3[fb], in1=ps)
            dma_rr[fb % 3].dma_start(
                out=out_r[fb][:, (NCH - 1) * CH:NCH * CH], in_=os3[fb]
            )

    nc.finalize()
    return nc


def _in_maps(x, qkv_w, qkv_b, proj_w):
    import ml_dtypes

    bf = ml_dtypes.bfloat16
    maps = []
    for c in range(NCORE):
        b, hg = c // 4, c % 4
        fs = slice(hg * F, (hg + 1) * F)
        q, k = qkv_w[fs], qkv_w[DIM:][fs]
        # device column order [k01 | q01 | k23 | q23]
        wqk = np.concatenate([k[:128], q[:128], k[128:], q[128:]], 0)  # [512,1024]
        bqk = np.concatenate([qkv_b[fs], qkv_b[DIM:][fs]], 0)
        maps.append({
            "xt": np.ascontiguousarray(x[b].T).astype(bf),
            "wqk": np.ascontiguousarray(wqk.T).astype(bf),
            "wv": np.ascontiguousarray(qkv_w[2 * DIM:][fs].T).astype(bf),
            "bqk": np.ascontiguousarray(bqk),
            "bv": np.ascontiguousarray(qkv_b[2 * DIM:][fs]),
            "pw": np.ascontiguousarray(proj_w[:, fs].T).astype(bf),
        })
    return maps


def _run(inputs, trace=False, trace_kwargs=None):
    from concourse.bass_utils import run_bass_kernel_spmd

    if "nc" not in _cache:
        _cache["nc"] = _build()
    nc = _cache["nc"]
    maps = _in_maps(inputs["x"], inputs["qkv_w"], inputs["qkv_b"], inputs["proj_w"])
    res = run_bass_kernel_spmd(
        nc, maps, list(range(NCORE)), trace=trace, **(trace_kwargs or {})
    )
    outs = [r["out"] for r in res.results]              # [1024, 2048] partials
    full = np.empty((B, N, DIM), dtype=np.float32)
    for b in range(B):
        acc = outs[4 * b].copy()
        for c in range(4 * b + 1, 4 * b + 4):
            acc += outs[c]
        full[b] = acc.T + inputs["proj_b"]
    return full, res


def kernel(**inputs) -> np.ndarray:
    out, _ = _run(inputs, trace=False)
    return out


# revision 40
# speedup vs baseline: 1.1316x; 1.0074x over previous
"""Multi-head attention (B=2, N=2048, D=1024, H=16) on 8 TRN2 NeuronCores.

Sharding: core c handles batch b=c//4 and head group hg=c%4 (4 heads of 16).
Each core computes QKV for its heads, materialized attention, and a partial
projection (proj row-split over heads); the host sums 4 partials per batch
and adds proj bias.  No device collectives.

v2 schedule, engineered to the PE roofline (~136.5us of moving-row time):
  - chunk-granular input DMA (one descriptor-batch per x chunk) spread over
    4 queues so the first score matmul fires at ~6us
  - hp-interleaved unit order (0,0),(1,0),(0,1),(1,1),... so projection
    work for chunk cc unlocks right after unit (1,cc) and spreads forward
  - exp split: most kb-slabs on ACT (hardware Exp), kbs in DVE_KBS per unit
    computed on the Vector engine with a Schraudolph fast-exp (scores*A+B
    -> int16 -> bitcast bf16), keeping ACT under the PE floor
  - V bias-add + normalize-mul + proj-psum drain on Pool, score bias +
    AV-psum drain + reciprocal on Vector: no engine above ~60% of the span
  - PSUM: 4 banks score double-buffer, 2 banks AV accumulators, 2 banks
    shared QKV/V/proj staging (prologue QK groups borrow the score banks)
"""

import numpy as np

B, N, DIM, H, DH = 2, 2048, 1024, 16, 64
SCALE = DH ** -0.5
NCORE = 8
HPC = 4            # heads per core
F = HPC * DH       # 256 features per core-headgroup
CH = 512           # token chunk (matmul moving free dim)
NCH = N // CH      # 4
KT = DIM // 128    # 8 k-tiles over model dim
TB = N // 128      # 16 token blocks
DVE_KBS = (1, 4, 7, 10, 13)  # kb slabs per unit whose exp runs on DVE (fast-exp)
K_SHIFT = 2.5      # global score shift: E = exp(s - K), cancels in softmax,
                   # keeps exp values in fp8-e4m3 range (row maxes are ~0.8-3.1)
EXP8_A = SCALE * 8.0 / float(np.log(2.0))          # schraudolph->e4m3 multiplier
EXP8_B = 7.0 * 8.0 - 0.44 - K_SHIFT * 8.0 / float(np.log(2.0))
_cache = {}


def _build():
    from contextlib import ExitStack

    import concourse.mybir as mybir
    from concourse import bacc
    from concourse.tile import TileContext

    f32 = mybir.dt.float32
    bf16 = mybir.dt.bfloat16
    f8 = mybir.dt.float8e4
    u8 = mybir.dt.uint8
    nc = bacc.Bacc("TRN2", target_bir_lowering=False)

    xt_d = nc.declare_dram_parameter("xt", [DIM, N], bf16, isOutput=False)
    wqk_d = nc.declare_dram_parameter("wqk", [DIM, 2 * F], bf16, isOutput=False)
    wv_d = nc.declare_dram_parameter("wv", [DIM, F], bf16, isOutput=False)
    bqk_d = nc.declare_dram_parameter("bqk", [2 * F], f32, isOutput=False)
    bv_d = nc.declare_dram_parameter("bv", [F], f32, isOutput=False)
    pw_d = nc.declare_dram_parameter("pw", [F, DIM], bf16, isOutput=False)
    out_d = nc.declare_dram_parameter("out", [DIM, N], bf16, isOutput=True)
    rscr = nc.dram_tensor("rscr", [2, NCH, 2 * CH], f32)

    # chunk-major views: one DMA delivers [128, 8, *] (all 8 k-tiles)
    xt_r = xt_d.ap().rearrange("(t p) n -> p t n", p=128)
    wqk_r = wqk_d.ap().rearrange("(t p) m -> p t m", p=128)
    wv_r = wv_d.ap().rearrange("(t p) m -> p t m", p=128)
    pw_r = pw_d.ap().rearrange("(t p) m -> t p m", p=128)
    out_r = out_d.ap().rearrange("(t p) n -> t p n", p=128)

    with TileContext(nc) as tc, ExitStack() as st:
        consts = st.enter_context(tc.tile_pool(name="consts", bufs=1))
        qkp = st.enter_context(tc.tile_pool(name="qkp", bufs=1))
        vtp = st.enter_context(tc.tile_pool(name="vtp", bufs=1))
        otp = st.enter_context(tc.tile_pool(name="otp", bufs=1))
        ep = st.enter_context(tc.tile_pool(name="ep", bufs=2))
        recp = st.enter_context(tc.tile_pool(name="recp", bufs=2))
        outs = st.enter_context(tc.tile_pool(name="outs", bufs=3))
        stgp = st.enter_context(tc.tile_pool(name="stgp", bufs=2))
        xw = st.enter_context(tc.tile_pool(name="xw", bufs=1))
        ps_mm = st.enter_context(tc.tile_pool(name="ps_mm", bufs=2, space="PSUM"))
        ps_s = st.enter_context(tc.tile_pool(name="ps_s", bufs=2, space="PSUM"))
        ps_av = st.enter_context(tc.tile_pool(name="ps_av", bufs=2, space="PSUM"))

        # ---- constant + weight tiles -------------------------------------
        bqk_sb = consts.tile([128, 2 * F // 128], f32)
        bv_sb = consts.tile([128, F], f32)
        nks_sb = consts.tile([128, 1], f32)
        nc.vector.memset(nks_sb, -K_SHIFT)
        # host supplies wqk with columns reordered to [k01 | q01 | k23 | q23]
        wkq01_sb = xw.tile([128, KT, F], bf16)
        wkq23_sb = xw.tile([128, KT, F], bf16)
        wv_sb = xw.tile([128, KT, F], bf16)
        pw_sb = [consts.tile([128, DIM], bf16, tag=f"pw{t}", name=f"pw{t}")
                 for t in range(2)]
        xt_sb = [xw.tile([128, KT, CH], bf16, tag=f"x{ch}", name=f"x{ch}")
                 for ch in range(NCH)]

        # DMA plan (issue ~0.6us fixed, ~150GB/s per ring, keep elements
        # >=512B).  Ring loads: sync: kq01, x0b, x1 | scalar: wv, kq23,
        # x3, pw | gpsimd: x0a, biases, x2.  k01+q01+x0 gate the first
        # score matmul.
        nc.sync.dma_start(out=wkq01_sb[:, :4, :], in_=wqk_r[:, :4, :F])
        nc.gpsimd.dma_start(out=xt_sb[0][:, :4, :], in_=xt_r[:, :4, 0 * CH:1 * CH])
        nc.scalar.dma_start(out=wv_sb, in_=wv_r)
        nc.sync.dma_start(out=wkq01_sb[:, 4:, :], in_=wqk_r[:, 4:, :F])
        nc.gpsimd.dma_start(out=bqk_sb, in_=bqk_d.ap().rearrange("(f p) -> p f", p=128))
        nc.gpsimd.dma_start(out=bv_sb, in_=bv_d.ap().partition_broadcast(128))
        nc.gpsimd.dma_start(out=xt_sb[0][:, 4:, :], in_=xt_r[:, 4:, 0 * CH:1 * CH])
        nc.scalar.dma_start(out=wkq23_sb, in_=wqk_r[:, :, F:])
        nc.sync.dma_start(out=xt_sb[1], in_=xt_r[:, :, 1 * CH:2 * CH])
        nc.gpsimd.dma_start(out=xt_sb[2], in_=xt_r[:, :, 2 * CH:3 * CH])
        nc.scalar.dma_start(out=xt_sb[3], in_=xt_r[:, :, 3 * CH:4 * CH])
        for t in range(2):
            nc.scalar.dma_start(out=pw_sb[t], in_=pw_r[t])

        # ---- working tiles ----------------------------------------------
        qk_sb = [[qkp.tile([128, CH], bf16, tag=f"qk{fb}_{ch}", name=f"qk{fb}_{ch}")
                  for ch in range(NCH)] for fb in range(4)]
        # fp8 V, two token-blocks per tile (the DoubleRow k-subtile pair)
        vt_sb = [vtp.tile([128, 2, HPC, DH + 4], f8, tag=f"vt{tb2}", name=f"vt{tb2}")
                 for tb2 in range(TB // 2)]
        ot_sb = [[otp.tile([128, CH], bf16, tag=f"ot{t}_{ch}", name=f"ot{t}_{ch}")
                  for ch in range(NCH)] for t in range(2)]

        def emit_qk_group(fb, ch, pool):
            # fb: 0=q01 1=q23 2=k01 3=k23; host column order [k01 q01 k23 q23]
            w = wkq01_sb if fb % 2 == 0 else wkq23_sb
            wo = 0 if fb >= 2 else 128
            ps = pool.tile([128, CH], f32, tag=pool.name.startswith("ps_s") and "sp" or "mm",
                           name=f"qkg{fb}_{ch}")
            for t in range(KT):
                nc.tensor.matmul(
                    ps,
                    w[:, t, wo:wo + 128],
                    xt_sb[ch][:, t, :],
                    start=(t == 0), stop=(t == KT - 1),
                )
            nc.scalar.activation(
                out=qk_sb[fb][ch], in_=ps,
                func=mybir.ActivationFunctionType.Identity,
                bias=bqk_sb[:, fb:fb + 1],
            )

        def emit_v_group(tb):
            # prologue-only: borrows the (still unused) AV psum slots
            ps = ps_av.tile([128, F], f32, tag="av", name=f"vg{tb}")
            ch, blk = tb // 4, tb % 4
            for t in range(KT):
                nc.tensor.matmul(
                    ps,
                    xt_sb[ch][:, t, blk * 128:(blk + 1) * 128],
                    wv_sb[:, t, :],
                    start=(t == 0), stop=(t == KT - 1),
                )
            nc.vector.tensor_add(
                out=vt_sb[tb // 2][:, tb % 2, :, :DH],
                in0=ps.rearrange("p (h d) -> p h d", h=HPC),
                in1=bv_sb.rearrange("p (h d) -> p h d", h=HPC),
            )
            nc.vector.memset(vt_sb[tb // 2][:, tb % 2, :, DH:], 0.0)
            nc.vector.memset(vt_sb[tb // 2][:, tb % 2, :, DH:DH + 1], 1.0)

        # ---- attention units: unit = (head-pair hp, chunk cc) ------------
        units = [(hp, cc) for cc in range(NCH) for hp in (0, 1)]
        et_store = {}

        def q_slice(h, cc):
            return qk_sb[h // 2][cc][(h % 2) * 64:(h % 2) * 64 + 64, :]

        def k_slice(h, kb):
            t = qk_sb[2 + h // 2][kb // 4]
            return t[(h % 2) * 64:(h % 2) * 64 + 64, (kb % 4) * 128:(kb % 4 + 1) * 128]

        def emit_s(u, kb):
            # scores for slab kb; exp -> fp8 half of the paired E tile
            hp, cc = u
            sp = ps_s.tile([128, 2 * CH], f32, tag="sp", name=f"sp{hp}_{cc}_{kb}")
            for j in range(2):
                h = 2 * hp + j
                nc.tensor.matmul(
                    sp[:, j * CH:(j + 1) * CH],
                    k_slice(h, kb),
                    q_slice(h, cc),
                    start=True, stop=True,
                )
            kb2 = kb // 2
            if kb % 2 == 0:
                e = ep.tile([128, 2, 2 * CH], f8, tag=f"e{kb2}",
                            name=f"e{hp}_{cc}_{kb2}")
                et_store[u][kb2] = e
            else:
                e = et_store[u][kb2]
            eh = e[:, kb % 2, :]
            if kb in DVE_KBS:
                nc.vector.tensor_scalar(
                    out=eh.bitcast(u8), in0=sp,
                    scalar1=float(EXP8_A), scalar2=float(EXP8_B),
                    op0=mybir.AluOpType.mult, op1=mybir.AluOpType.add,
                )
            else:
                nc.scalar.activation(
                    out=eh, in_=sp,
                    func=mybir.ActivationFunctionType.Exp, scale=SCALE,
                    bias=nks_sb[:, 0:1],
                )
            et_store[u][kb2] = e

        dma_rr = [nc.sync, nc.gpsimd, nc.scalar]

        def proj_ops(fb, cc):
            # projection group split into two single-matmul filler ops; the
            # psum drain runs on the (slack) ACT engine
            state = {}

            def op1():
                ps = ps_mm.tile([128, CH], f32, tag="mm", name=f"pj{fb}_{cc}")
                state["ps"] = ps
                nc.tensor.matmul(ps, pw_sb[0][:, fb * 128:(fb + 1) * 128],
                                 ot_sb[0][cc], start=True, stop=False)

            def op2():
                ps = state["ps"]
                nc.tensor.matmul(ps, pw_sb[1][:, fb * 128:(fb + 1) * 128],
                                 ot_sb[1][cc], start=False, stop=True)
                os = outs.tile([128, CH], bf16, tag="os", name=f"os{fb}_{cc}")
                nc.vector.tensor_copy(out=os, in_=ps)
                dma_rr[fb % 3].dma_start(
                    out=out_r[fb][:, cc * CH:(cc + 1) * CH], in_=os
                )

            return [("proj", op1), ("proj", op2)]

        def qk_ops(fb, ch):
            # QK group as 8 single-matmul ops + bias drain on the last
            w = wkq01_sb if fb % 2 == 0 else wkq23_sb
            wo = 0 if fb >= 2 else 128
            state = {}
            ops = []

            def mk(t):
                def op():
                    if t == 0:
                        state["ps"] = ps_mm.tile([128, CH], f32, tag="mm",
                                                 name=f"qkg{fb}_{ch}")
                    nc.tensor.matmul(state["ps"], w[:, t, wo:wo + 128],
                                     xt_sb[ch][:, t, :],
                                     start=(t == 0), stop=(t == KT - 1))
                    if t == KT - 1:
                        nc.scalar.activation(
                            out=qk_sb[fb][ch], in_=state["ps"],
                            func=mybir.ActivationFunctionType.Identity,
                            bias=bqk_sb[:, fb:fb + 1])
                return op

            for t in range(KT):
                ops.append(("qk", mk(t)))
            return ops

        # ---- prologue: k01/q01 chunk0, then unit-0 scores + V fillers ----
        emit_qk_group(2, 0, ps_s)   # k01 c0 (borrows a score bank)
        emit_qk_group(0, 0, ps_s)   # q01 c0
        et_store[units[0]] = [None] * (TB // 2)
        # per-kb filler: one V group per slab; k01(c+1) before S needs it;
        # k23/q23 chunk0 early so unit (1,0)'s score stream can start.
        pro_fill = {1: [(3, 0, ps_mm)], 3: [(2, 1, ps_mm)], 5: [(1, 0, ps_mm)],
                    7: [(2, 2, ps_mm)], 11: [(2, 3, ps_mm)]}
        for kb in range(TB):
            emit_s(units[0], kb)
            emit_v_group(kb)
            for fb, ch, pool in pro_fill.get(kb, ()):
                emit_qk_group(fb, ch, pool)

        # filler op queues per unit: QK groups feed the stream two units
        # ahead; proj drains chunks closed by the preceding (1,*) unit.
        fillq = {
            0: qk_ops(3, 1) + qk_ops(3, 2) + qk_ops(3, 3) + qk_ops(0, 1),
            1: qk_ops(1, 1),
            2: qk_ops(0, 2) + sum((proj_ops(fb, 0) for fb in range(KT)), []),
            3: qk_ops(1, 2),
            4: qk_ops(0, 3) + sum((proj_ops(fb, 1) for fb in range(KT)), []),
            5: qk_ops(1, 3),
            6: (proj_ops(0, 2) + proj_ops(1, 2) + proj_ops(2, 2)
                + proj_ops(3, 2) + proj_ops(4, 2) + proj_ops(5, 2)
                + proj_ops(6, 2) + proj_ops(7, 2)),
        }
        os3 = []

        def os3_prefill(fb):
            def op():
                ps = ps_mm.tile([128, CH], f32, tag="mm", name=f"pj3a{fb}")
                nc.tensor.matmul(ps, pw_sb[0][:, fb * 128:(fb + 1) * 128],
                                 ot_sb[0][NCH - 1], start=True, stop=True)
                os = outs.tile([128, CH], bf16, tag=f"os3_{fb}", bufs=1,
                               name=f"os3_{fb}")
                nc.vector.tensor_copy(out=os, in_=ps)
                os3.append(os)
            return ("proj", op)

        fillq[7] = [os3_prefill(fb) for fb in range(KT)]

        for i, u in enumerate(units):
            hp, cc = u
            nxt = units[i + 1] if i + 1 < len(units) else None
            if nxt is not None:
                et_store[nxt] = [None] * (TB // 2)
            avs = [
                ps_av.tile([68, CH], f32, tag="av", name=f"av{hp}_{cc}_{j}")
                for j in range(2)
            ]
            ops = fillq.get(i, [])
            for kb in range(TB):
                if nxt is not None:
                    emit_s(nxt, kb)
                # one fp8 DoubleRow AV matmul per slot (2 k-subtiles each):
                # head j = kb%2 over token-block pair kb2 = kb//2
                j, kb2 = kb % 2, kb // 2
                nc.tensor.matmul(
                    avs[j],
                    vt_sb[kb2][:, :, 2 * hp + j, :],
                    et_store[u][kb2][:, :, j * CH:(j + 1) * CH],
                    start=(kb2 == 0), stop=(kb2 == TB // 2 - 1),
                    perf_mode=mybir.MatmulPerfMode.DoubleRow,
                )
                # pop fillers: enough each slot to drain the queue by unit end
                npop = -(-len(ops) // (TB - kb))
                for _ in range(min(npop, 3)):
                    if ops:
                        ops.pop(0)[1]()
            for kind, op in ops:
                op()
            et_store.pop(u)

            # epilogue: drain AV psum to SBUF (frees banks), reciprocal of
            # the ones-row, broadcast via DRAM round-trip, normalize on Pool
            stg = stgp.tile([65, 2 * CH], f32, tag="stg", name=f"stg{hp}_{cc}")
            sums = recp.tile([1, 2 * CH], f32, tag="sums", name=f"sums{hp}_{cc}")
            for j in range(2):
                nc.vector.tensor_copy(out=stg[:, j * CH:(j + 1) * CH], in_=avs[j][:65, :])
                nc.vector.tensor_copy(
                    out=sums[:, j * CH:(j + 1) * CH], in_=avs[j][64:65, :]
                )
            rec = recp.tile([1, 2 * CH], f32, tag="rec", name=f"rec{hp}_{cc}")
            nc.vector.reciprocal_approx_fast(out=rec, in_=sums)
            nc.gpsimd.dma_start(out=rscr.ap()[hp, cc], in_=rec)
            rec64 = recp.tile([64, 2 * CH], f32, tag="rec64", name=f"rb{hp}_{cc}")
            nc.gpsimd.dma_start(
                out=rec64, in_=rscr.ap()[hp, cc].partition_broadcast(64)
            )
            for j in range(2):
                h = 2 * hp + j
                nc.gpsimd.tensor_mul(
                    out=ot_sb[h // 2][cc][(h % 2) * 64:(h % 2) * 64 + 64, :],
                    in0=stg[0:64, j * CH:(j + 1) * CH],
                    in1=rec64[:, j * CH:(j + 1) * CH],
                )
        # chunk-3 projection tail: add the t=1 half onto the prefilled t=0
        for fb in range(KT):
            ps = ps_mm.tile([128, CH], f32, tag="mm", name=f"pj3b{fb}")
            nc.tensor.matmul(ps, pw_sb[1][:, fb * 128:(fb + 1) * 128],
                             ot_sb[1][NCH - 1], start=True, stop=True)
            nc.vector.tensor_add(out=os3[fb], in0=os3[fb], in1=ps)
            dma_rr[fb % 3].dma_start(
                out=out_r[fb][:, (NCH - 1) * CH:NCH * CH], in_=os3[fb]
            )

    nc.finalize()
    return nc


def _in_maps(x, qkv_w, qkv_b, proj_w):
    import ml_dtypes

    bf = ml_dtypes.bfloat16
    maps = []
    for c in range(NCORE):
        b, hg = c // 4, c % 4
        fs = slice(hg * F, (hg + 1) * F)
        q, k = qkv_w[fs], qkv_w[DIM:][fs]
        # device column order [k01 | q01 | k23 | q23]
        wqk = np.concatenate([k[:128], q[:128], k[128:], q[128:]], 0)  # [512,1024]
        bqk = np.concatenate([qkv_b[fs], qkv_b[DIM:][fs]], 0)
        maps.append({
            "xt": np.ascontiguousarray(x[b].T).astype(bf),
            "wqk": np.ascontiguousarray(wqk.T).astype(bf),
            "wv": np.ascontiguousarray(qkv_w[2 * DIM:][fs].T).astype(bf),
            "bqk": np.ascontiguousarray(bqk),
            "bv": np.ascontiguousarray(qkv_b[2 * DIM:][fs]),
            "pw": np.ascontiguousarray(proj_w[:, fs].T).astype(bf),
        })
    return maps


def _run(inputs, trace=False, trace_kwargs=None):
    from concourse.bass_utils import run_bass_kernel_spmd

    if "nc" not in _cache:
        _cache["nc"] = _build()
    nc = _cache["nc"]
    maps = _in_maps(inputs["x"], inputs["qkv_w"], inputs["qkv_b"], inputs["proj_w"])
    res = run_bass_kernel_spmd(
        nc, maps, list(range(NCORE)), trace=trace, **(trace_kwargs or {})
    )
    outs = [r["out"] for r in res.results]              # [1024, 2048] bf16 partials
    full = np.empty((B, N, DIM), dtype=np.float32)
    for b in range(B):
        acc = outs[4 * b].astype(np.float32)
        for c in range(4 * b + 1, 4 * b + 4):
            acc += outs[c].astype(np.float32)
        full[b] = acc.T + inputs["proj_b"]
    return full, res


def kernel(**inputs) -> np.ndarray:
    out, _ = _run(inputs, trace=False)
    return out


# revision 46
# speedup vs baseline: 1.1370x; 1.0048x over previous
"""Multi-head attention (B=2, N=2048, D=1024, H=16) on 8 TRN2 NeuronCores.

Sharding: core c handles batch b=c//4 and head group hg=c%4 (4 heads of 16).
Each core computes QKV for its heads, materialized attention, and a partial
projection (proj row-split over heads); the host sums 4 partials per batch
and adds proj bias.  No device collectives.

v2 schedule, engineered to the PE roofline (~136.5us of moving-row time):
  - chunk-granular input DMA (one descriptor-batch per x chunk) spread over
    4 queues so the first score matmul fires at ~6us
  - hp-interleaved unit order (0,0),(1,0),(0,1),(1,1),... so projection
    work for chunk cc unlocks right after unit (1,cc) and spreads forward
  - exp split: most kb-slabs on ACT (hardware Exp), kbs in DVE_KBS per unit
    computed on the Vector engine with a Schraudolph fast-exp (scores*A+B
    -> int16 -> bitcast bf16), keeping ACT under the PE floor
  - V bias-add + normalize-mul + proj-psum drain on Pool, score bias +
    AV-psum drain + reciprocal on Vector: no engine above ~60% of the span
  - PSUM: 4 banks score double-buffer, 2 banks AV accumulators, 2 banks
    shared QKV/V/proj staging (prologue QK groups borrow the score banks)
"""

import numpy as np

B, N, DIM, H, DH = 2, 2048, 1024, 16, 64
SCALE = DH ** -0.5
NCORE = 8
HPC = 4            # heads per core
F = HPC * DH       # 256 features per core-headgroup
CH = 512           # token chunk (matmul moving free dim)
NCH = N // CH      # 4
KT = DIM // 128    # 8 k-tiles over model dim
TB = N // 128      # 16 token blocks
DVE_KBS = (1, 4, 7, 10, 13)  # kb slabs per unit whose exp runs on DVE (fast-exp)
K_SHIFT = 2.5      # global score shift: E = exp(s - K), cancels in softmax,
                   # keeps exp values in fp8-e4m3 range (row maxes are ~0.8-3.1)
EXP8_A = SCALE * 8.0 / float(np.log(2.0))          # schraudolph->e4m3 multiplier
EXP8_B = 7.0 * 8.0 - 0.44 - K_SHIFT * 8.0 / float(np.log(2.0))
_cache = {}


def _build():
    from contextlib import ExitStack

    import concourse.mybir as mybir
    from concourse import bacc
    from concourse.tile import TileContext

    f32 = mybir.dt.float32
    bf16 = mybir.dt.bfloat16
    f8 = mybir.dt.float8e4
    u8 = mybir.dt.uint8
    nc = bacc.Bacc("TRN2", target_bir_lowering=False)

    xt_d = nc.declare_dram_parameter("xt", [DIM, N], bf16, isOutput=False)
    wqk_d = nc.declare_dram_parameter("wqk", [DIM, 2 * F], bf16, isOutput=False)
    wv_d = nc.declare_dram_parameter("wv", [DIM, F], bf16, isOutput=False)
    bqk_d = nc.declare_dram_parameter("bqk", [2 * F], f32, isOutput=False)
    bv_d = nc.declare_dram_parameter("bv", [F], f32, isOutput=False)
    pw_d = nc.declare_dram_parameter("pw", [F, DIM], bf16, isOutput=False)
    out_d = nc.declare_dram_parameter("out", [DIM, N], bf16, isOutput=True)
    rscr = nc.dram_tensor("rscr", [2, NCH, 2 * CH], f32)

    # chunk-major views: one DMA delivers [128, 8, *] (all 8 k-tiles)
    xt_r = xt_d.ap().rearrange("(t p) n -> p t n", p=128)
    wqk_r = wqk_d.ap().rearrange("(t p) m -> p t m", p=128)
    wv_r = wv_d.ap().rearrange("(t p) m -> p t m", p=128)
    pw_r = pw_d.ap().rearrange("(t p) m -> t p m", p=128)
    out_r = out_d.ap().rearrange("(t p) n -> t p n", p=128)

    with TileContext(nc) as tc, ExitStack() as st:
        consts = st.enter_context(tc.tile_pool(name="consts", bufs=1))
        qkp = st.enter_context(tc.tile_pool(name="qkp", bufs=1))
        vtp = st.enter_context(tc.tile_pool(name="vtp", bufs=1))
        otp = st.enter_context(tc.tile_pool(name="otp", bufs=1))
        ep = st.enter_context(tc.tile_pool(name="ep", bufs=2))
        recp = st.enter_context(tc.tile_pool(name="recp", bufs=2))
        outs = st.enter_context(tc.tile_pool(name="outs", bufs=3))
        stgp = st.enter_context(tc.tile_pool(name="stgp", bufs=2))
        xw = st.enter_context(tc.tile_pool(name="xw", bufs=1))
        ps_mm = st.enter_context(tc.tile_pool(name="ps_mm", bufs=2, space="PSUM"))
        ps_s = st.enter_context(tc.tile_pool(name="ps_s", bufs=2, space="PSUM"))
        ps_av = st.enter_context(tc.tile_pool(name="ps_av", bufs=2, space="PSUM"))

        # ---- constant + weight tiles -------------------------------------
        bqk_sb = consts.tile([128, 2 * F // 128], f32)
        bv_sb = consts.tile([128, F], f32)
        nks_sb = consts.tile([128, 1], f32)
        nc.vector.memset(nks_sb, -K_SHIFT)
        # host supplies wqk with columns reordered to [k01 | q01 | k23 | q23]
        wkq01_sb = xw.tile([128, KT, F], bf16)
        wkq23_sb = xw.tile([128, KT, F], bf16)
        wv_sb = xw.tile([128, KT, F], bf16)
        pw_sb = [consts.tile([128, DIM], bf16, tag=f"pw{t}", name=f"pw{t}")
                 for t in range(2)]
        xt_sb = [xw.tile([128, KT, CH], bf16, tag=f"x{ch}", name=f"x{ch}")
                 for ch in range(NCH)]

        # DMA plan (issue ~0.6us fixed, ~150GB/s per ring, keep elements
        # >=512B).  Ring loads: sync: kq01, x0b, x1 | scalar: wv, kq23,
        # x3, pw | gpsimd: x0a, biases, x2.  k01+q01+x0 gate the first
        # score matmul.
        nc.sync.dma_start(out=wkq01_sb[:, :4, :], in_=wqk_r[:, :4, :F])
        nc.gpsimd.dma_start(out=xt_sb[0][:, :4, :], in_=xt_r[:, :4, 0 * CH:1 * CH])
        nc.scalar.dma_start(out=wv_sb, in_=wv_r)
        nc.sync.dma_start(out=wkq01_sb[:, 4:, :], in_=wqk_r[:, 4:, :F])
        nc.gpsimd.dma_start(out=bqk_sb, in_=bqk_d.ap().rearrange("(f p) -> p f", p=128))
        nc.gpsimd.dma_start(out=bv_sb, in_=bv_d.ap().partition_broadcast(128))
        nc.gpsimd.dma_start(out=xt_sb[0][:, 4:, :], in_=xt_r[:, 4:, 0 * CH:1 * CH])
        nc.scalar.dma_start(out=wkq23_sb, in_=wqk_r[:, :, F:])
        nc.sync.dma_start(out=xt_sb[1], in_=xt_r[:, :, 1 * CH:2 * CH])
        nc.gpsimd.dma_start(out=xt_sb[2], in_=xt_r[:, :, 2 * CH:3 * CH])
        nc.scalar.dma_start(out=xt_sb[3], in_=xt_r[:, :, 3 * CH:4 * CH])
        for t in range(2):
            nc.scalar.dma_start(out=pw_sb[t], in_=pw_r[t])

        # ---- working tiles ----------------------------------------------
        qk_sb = [[qkp.tile([128, CH], bf16, tag=f"qk{fb}_{ch}", name=f"qk{fb}_{ch}")
                  for ch in range(NCH)] for fb in range(4)]
        # fp8 V, two token-blocks per tile (the DoubleRow k-subtile pair)
        vt_sb = [vtp.tile([128, 2, HPC, DH + 4], f8, tag=f"vt{tb2}", name=f"vt{tb2}")
                 for tb2 in range(TB // 2)]
        ot_sb = [[otp.tile([128, CH], bf16, tag=f"ot{t}_{ch}", name=f"ot{t}_{ch}")
                  for ch in range(NCH)] for t in range(2)]

        def emit_qk_group(fb, ch, pool, c0=0, c1=CH):
            # fb: 0=q01 1=q23 2=k01 3=k23; host column order [k01 q01 k23 q23]
            w = wkq01_sb if fb % 2 == 0 else wkq23_sb
            wo = 0 if fb >= 2 else 128
            ps = pool.tile([128, c1 - c0], f32,
                           tag=pool.name.startswith("ps_s") and "sp" or "mm",
                           name=f"qkg{fb}_{ch}")
            for t in range(KT):
                nc.tensor.matmul(
                    ps,
                    w[:, t, wo:wo + 128],
                    xt_sb[ch][:, t, c0:c1],
                    start=(t == 0), stop=(t == KT - 1),
                )
            nc.scalar.activation(
                out=qk_sb[fb][ch][:, c0:c1], in_=ps,
                func=mybir.ActivationFunctionType.Identity,
                bias=bqk_sb[:, fb:fb + 1],
            )

        def emit_v_group(tb):
            # prologue-only: borrows the (still unused) AV psum slots
            ps = ps_av.tile([128, F], f32, tag="av", name=f"vg{tb}")
            ch, blk = tb // 4, tb % 4
            for t in range(KT):
                nc.tensor.matmul(
                    ps,
                    xt_sb[ch][:, t, blk * 128:(blk + 1) * 128],
                    wv_sb[:, t, :],
                    start=(t == 0), stop=(t == KT - 1),
                )
            nc.vector.tensor_add(
                out=vt_sb[tb // 2][:, tb % 2, :, :DH],
                in0=ps.rearrange("p (h d) -> p h d", h=HPC),
                in1=bv_sb.rearrange("p (h d) -> p h d", h=HPC),
            )
            nc.vector.memset(vt_sb[tb // 2][:, tb % 2, :, DH:], 0.0)
            nc.vector.memset(vt_sb[tb // 2][:, tb % 2, :, DH:DH + 1], 1.0)

        # ---- attention units: unit = (head-pair hp, chunk cc) ------------
        units = [(hp, cc) for cc in range(NCH) for hp in (0, 1)]
        et_store = {}

        def q_slice(h, cc):
            return qk_sb[h // 2][cc][(h % 2) * 64:(h % 2) * 64 + 64, :]

        def k_slice(h, kb):
            t = qk_sb[2 + h // 2][kb // 4]
            return t[(h % 2) * 64:(h % 2) * 64 + 64, (kb % 4) * 128:(kb % 4 + 1) * 128]

        def emit_s(u, kb):
            # scores for slab kb; exp -> fp8 half of the paired E tile
            hp, cc = u
            sp = ps_s.tile([128, 2 * CH], f32, tag="sp", name=f"sp{hp}_{cc}_{kb}")
            for j in range(2):
                h = 2 * hp + j
                nc.tensor.matmul(
                    sp[:, j * CH:(j + 1) * CH],
                    k_slice(h, kb),
                    q_slice(h, cc),
                    start=True, stop=True,
                )
            kb2 = kb // 2
            if kb % 2 == 0:
                e = ep.tile([128, 2, 2 * CH], f8, tag=f"e{kb2}",
                            name=f"e{hp}_{cc}_{kb2}")
                et_store[u][kb2] = e
            else:
                e = et_store[u][kb2]
            eh = e[:, kb % 2, :]
            if kb in DVE_KBS:
                nc.vector.tensor_scalar(
                    out=eh.bitcast(u8), in0=sp,
                    scalar1=float(EXP8_A), scalar2=float(EXP8_B),
                    op0=mybir.AluOpType.mult, op1=mybir.AluOpType.add,
                )
            else:
                nc.scalar.activation(
                    out=eh, in_=sp,
                    func=mybir.ActivationFunctionType.Exp, scale=SCALE,
                    bias=nks_sb[:, 0:1],
                )
            et_store[u][kb2] = e

        dma_rr = [nc.sync, nc.gpsimd, nc.scalar]

        def proj_ops(fb, cc):
            # projection group split into two single-matmul filler ops; the
            # psum drain runs on the (slack) ACT engine
            state = {}

            def op1():
                ps = ps_mm.tile([128, CH], f32, tag="mm", name=f"pj{fb}_{cc}")
                state["ps"] = ps
                nc.tensor.matmul(ps, pw_sb[0][:, fb * 128:(fb + 1) * 128],
                                 ot_sb[0][cc], start=True, stop=False)

            def op2():
                ps = state["ps"]
                nc.tensor.matmul(ps, pw_sb[1][:, fb * 128:(fb + 1) * 128],
                                 ot_sb[1][cc], start=False, stop=True)
                os = outs.tile([128, CH], bf16, tag="os", name=f"os{fb}_{cc}")
                if fb % 2:
                    nc.scalar.copy(out=os, in_=ps)
                else:
                    nc.vector.tensor_copy(out=os, in_=ps)
                dma_rr[fb % 3].dma_start(
                    out=out_r[fb][:, cc * CH:(cc + 1) * CH], in_=os
                )

            return [("proj", op1), ("proj", op2)]

        def qk_ops(fb, ch):
            # QK group as 8 single-matmul ops + bias drain on the last
            w = wkq01_sb if fb % 2 == 0 else wkq23_sb
            wo = 0 if fb >= 2 else 128
            state = {}
            ops = []

            def mk(t):
                def op():
                    if t == 0:
                        state["ps"] = ps_mm.tile([128, CH], f32, tag="mm",
                                                 name=f"qkg{fb}_{ch}")
                    nc.tensor.matmul(state["ps"], w[:, t, wo:wo + 128],
                                     xt_sb[ch][:, t, :],
                                     start=(t == 0), stop=(t == KT - 1))
                    if t == KT - 1:
                        nc.scalar.activation(
                            out=qk_sb[fb][ch], in_=state["ps"],
                            func=mybir.ActivationFunctionType.Identity,
                            bias=bqk_sb[:, fb:fb + 1])
                return op

            for t in range(KT):
                ops.append(("qk", mk(t)))
            return ops

        # ---- prologue ----------------------------------------------------
        # q01(c0) full + only kb0's k01 columns first so the score/exp
        # stream starts as early as possible; the two groups interleave in
        # t-halves to cover the x0 second-half DMA gap.  k01's remaining
        # columns follow right behind S(kb0).
        psq = ps_s.tile([128, CH], f32, tag="sp", name="q01g")
        psk = ps_s.tile([128, 128], f32, tag="sp", name="k01p1")
        for t in range(4):
            nc.tensor.matmul(psq, wkq01_sb[:, t, 128:], xt_sb[0][:, t, :],
                             start=(t == 0), stop=False)
        for t in range(4):
            nc.tensor.matmul(psk, wkq01_sb[:, t, :128], xt_sb[0][:, t, :128],
                             start=(t == 0), stop=False)
        for t in range(4, KT):
            nc.tensor.matmul(psq, wkq01_sb[:, t, 128:], xt_sb[0][:, t, :],
                             start=False, stop=(t == KT - 1))
        nc.scalar.activation(out=qk_sb[0][0], in_=psq,
                             func=mybir.ActivationFunctionType.Identity,
                             bias=bqk_sb[:, 0:1])
        for t in range(4, KT):
            nc.tensor.matmul(psk, wkq01_sb[:, t, :128], xt_sb[0][:, t, :128],
                             start=False, stop=(t == KT - 1))
        nc.scalar.activation(out=qk_sb[2][0][:, :128], in_=psk,
                             func=mybir.ActivationFunctionType.Identity,
                             bias=bqk_sb[:, 2:3])
        et_store[units[0]] = [None] * (TB // 2)
        # per-kb filler: one V group per slab; k01(c+1) before S needs it;
        # k23/q23 chunk0 early so unit (1,0)'s score stream can start.
        pro_fill = {0: [(2, 0, ps_mm, 128, CH)],
                    1: [(3, 0, ps_mm, 0, CH)], 3: [(2, 1, ps_mm, 0, CH)],
                    5: [(1, 0, ps_mm, 0, CH)], 7: [(2, 2, ps_mm, 0, CH)],
                    11: [(2, 3, ps_mm, 0, CH)]}
        for kb in range(TB):
            emit_s(units[0], kb)
            emit_v_group(kb)
            for fb, ch, pool, c0, c1 in pro_fill.get(kb, ()):
                emit_qk_group(fb, ch, pool, c0, c1)

        # filler op queues per unit: QK groups feed the stream two units
        # ahead; proj drains chunks closed by the preceding (1,*) unit.
        os3 = []
        fillq = {
            0: qk_ops(3, 1) + qk_ops(3, 2) + qk_ops(3, 3) + qk_ops(0, 1),
            1: qk_ops(1, 1),
            2: qk_ops(0, 2) + sum((proj_ops(fb, 0) for fb in range(KT)), []),
            3: qk_ops(1, 2),
            4: qk_ops(0, 3) + sum((proj_ops(fb, 1) for fb in range(KT)), []),
            5: qk_ops(1, 3),
            6: (proj_ops(0, 2) + proj_ops(1, 2) + proj_ops(2, 2)
                + proj_ops(3, 2) + proj_ops(4, 2) + proj_ops(5, 2)
                + proj_ops(6, 2) + proj_ops(7, 2)),
        }
        os3 = []

        # chunk-3 projection prefill: t=0 halves accumulate into open psum
        # groups parked in the freed mm/score banks; the tail then just adds
        # the t=1 half and drains once on the (idle) ACT engine.
        os3ps = {}

        def os3_prefill(fb):
            def op():
                if fb < 2:
                    ps = ps_mm.tile([128, CH], f32, tag="mm", name=f"pj3a{fb}")
                else:
                    sl = os3ps.get(("sp", (fb - 2) // 2))
                    if sl is None:
                        sl = ps_s.tile([128, 2 * CH], f32, tag="sp",
                                       name=f"pj3sp{(fb - 2) // 2}")
                        os3ps[("sp", (fb - 2) // 2)] = sl
                    ps = sl[:, (fb % 2) * CH:(fb % 2 + 1) * CH]
                os3ps[fb] = ps
                nc.tensor.matmul(ps, pw_sb[0][:, fb * 128:(fb + 1) * 128],
                                 ot_sb[0][NCH - 1], start=True, stop=False)
            return ("proj", op)

        fillq[7] = [os3_prefill(fb) for fb in range(6)]

        for i, u in enumerate(units):
            hp, cc = u
            nxt = units[i + 1] if i + 1 < len(units) else None
            if nxt is not None:
                et_store[nxt] = [None] * (TB // 2)
            avs = [
                ps_av.tile([68, CH], f32, tag="av", name=f"av{hp}_{cc}_{j}")
                for j in range(2)
            ]
            ops = fillq.get(i, [])
            for kb in range(TB):
                if nxt is not None:
                    emit_s(nxt, kb)
                # one fp8 DoubleRow AV matmul per slot (2 k-subtiles each):
                # head j = kb%2 over token-block pair kb2 = kb//2
                j, kb2 = kb % 2, kb // 2
                nc.tensor.matmul(
                    avs[j],
                    vt_sb[kb2][:, :, 2 * hp + j, :],
                    et_store[u][kb2][:, :, j * CH:(j + 1) * CH],
                    start=(kb2 == 0), stop=(kb2 == TB // 2 - 1),
                    perf_mode=mybir.MatmulPerfMode.DoubleRow,
                )
                # pop fillers: enough each slot to drain the queue by unit end
                npop = -(-len(ops) // (TB - kb))
                for _ in range(min(npop, 3)):
                    if ops:
                        ops.pop(0)[1]()
            for kind, op in ops:
                op()
            et_store.pop(u)

            # epilogue: drain AV psum to SBUF (frees banks), reciprocal of
            # the ones-row, broadcast via DRAM round-trip, normalize on Pool
            stg = stgp.tile([65, 2 * CH], f32, tag="stg", name=f"stg{hp}_{cc}")
            sums = recp.tile([1, 2 * CH], f32, tag="sums", name=f"sums{hp}_{cc}")
            for j in range(2):
                nc.vector.tensor_copy(out=stg[:, j * CH:(j + 1) * CH], in_=avs[j][:65, :])
                nc.vector.tensor_copy(
                    out=sums[:, j * CH:(j + 1) * CH], in_=avs[j][64:65, :]
                )
            rec = recp.tile([1, 2 * CH], f32, tag="rec", name=f"rec{hp}_{cc}")
            nc.vector.reciprocal_approx_fast(out=rec, in_=sums)
            nc.gpsimd.dma_start(out=rscr.ap()[hp, cc], in_=rec)
            rec64 = recp.tile([64, 2 * CH], f32, tag="rec64", name=f"rb{hp}_{cc}")
            nc.gpsimd.dma_start(
                out=rec64, in_=rscr.ap()[hp, cc].partition_broadcast(64)
            )
            for j in range(2):
                h = 2 * hp + j
                nc.gpsimd.tensor_mul(
                    out=ot_sb[h // 2][cc][(h % 2) * 64:(h % 2) * 64 + 64, :],
                    in0=stg[0:64, j * CH:(j + 1) * CH],
                    in1=rec64[:, j * CH:(j + 1) * CH],
                )
        # chunk-3 projection tail: close the prefilled groups, drain on ACT
        def os3_drain(fb):
            os = outs.tile([128, CH], bf16, tag=f"os3_{fb}", bufs=1,
                           name=f"os3_{fb}")
            nc.scalar.copy(out=os, in_=os3ps[fb])
            dma_rr[fb % 3].dma_start(
                out=out_r[fb][:, (NCH - 1) * CH:NCH * CH], in_=os
            )

        for fb in range(6):
            nc.tensor.matmul(os3ps[fb], pw_sb[1][:, fb * 128:(fb + 1) * 128],
                             ot_sb[1][NCH - 1], start=False, stop=True)
        os3_drain(0)
        os3_drain(1)
        for fb in range(6, KT):
            ps = ps_mm.tile([128, CH], f32, tag="mm", name=f"pj3b{fb}")
            os3ps[fb] = ps
            for t in range(2):
                nc.tensor.matmul(ps, pw_sb[t][:, fb * 128:(fb + 1) * 128],
                                 ot_sb[t][NCH - 1], start=(t == 0), stop=(t == 1))
        for fb in range(2, KT):
            os3_drain(fb)

    nc.finalize()
    return nc


def _in_maps(x, qkv_w, qkv_b, proj_w):
    import ml_dtypes

    bf = ml_dtypes.bfloat16
    maps = []
    for c in range(NCORE):
        b, hg = c // 4, c % 4
        fs = slice(hg * F, (hg + 1) * F)
        q, k = qkv_w[fs], qkv_w[DIM:][fs]
        # device column order [k01 | q01 | k23 | q23]
        wqk = np.concatenate([k[:128], q[:128], k[128:], q[128:]], 0)  # [512,1024]
        bqk = np.concatenate([qkv_b[fs], qkv_b[DIM:][fs]], 0)
        maps.append({
            "xt": np.ascontiguousarray(x[b].T).astype(bf),
            "wqk": np.ascontiguousarray(wqk.T).astype(bf),
            "wv": np.ascontiguousarray(qkv_w[2 * DIM:][fs].T).astype(bf),
            "bqk": np.ascontiguousarray(bqk),
            "bv": np.ascontiguousarray(qkv_b[2 * DIM:][fs]),
            "pw": np.ascontiguousarray(proj_w[:, fs].T).astype(bf),
        })
    return maps


def _run(inputs, trace=False, trace_kwargs=None):
    from concourse.bass_utils import run_bass_kernel_spmd

    if "nc" not in _cache:
        _cache["nc"] = _build()
    nc = _cache["nc"]
    maps = _in_maps(inputs["x"], inputs["qkv_w"], inputs["qkv_b"], inputs["proj_w"])
    res = run_bass_kernel_spmd(
        nc, maps, list(range(NCORE)), trace=trace, **(trace_kwargs or {})
    )
    outs = [r["out"] for r in res.results]              # [1024, 2048] bf16 partials
    full = np.empty((B, N, DIM), dtype=np.float32)
    for b in range(B):
        acc = outs[4 * b].astype(np.float32)
        for c in range(4 * b + 1, 4 * b + 4):
            acc += outs[c].astype(np.float32)
        full[b] = acc.T + inputs["proj_b"]
    return full, res


def kernel(**inputs) -> np.ndarray:
    out, _ = _run(inputs, trace=False)
    return out


# revision 48
# speedup vs baseline: 1.1944x; 1.0504x over previous
"""Multi-head attention (B=2, N=2048, D=1024, H=16) on 8 TRN2 NeuronCores.

Sharding: core c handles batch b=c//4 and head group hg=c%4 (4 heads of 16).
Each core computes QKV for its heads, materialized attention, and a partial
projection (proj row-split over heads); the host sums 4 partials per batch
and adds proj bias.  No device collectives.

v2 schedule, engineered to the PE roofline (~136.5us of moving-row time):
  - chunk-granular input DMA (one descriptor-batch per x chunk) spread over
    4 queues so the first score matmul fires at ~6us
  - hp-interleaved unit order (0,0),(1,0),(0,1),(1,1),... so projection
    work for chunk cc unlocks right after unit (1,cc) and spreads forward
  - exp split: most kb-slabs on ACT (hardware Exp), kbs in DVE_KBS per unit
    computed on the Vector engine with a Schraudolph fast-exp (scores*A+B
    -> int16 -> bitcast bf16), keeping ACT under the PE floor
  - V bias-add + normalize-mul + proj-psum drain on Pool, score bias +
    AV-psum drain + reciprocal on Vector: no engine above ~60% of the span
  - PSUM: 4 banks score double-buffer, 2 banks AV accumulators, 2 banks
    shared QKV/V/proj staging (prologue QK groups borrow the score banks)
"""

import numpy as np

B, N, DIM, H, DH = 2, 2048, 1024, 16, 64
SCALE = DH ** -0.5
NCORE = 8
HPC = 4            # heads per core
F = HPC * DH       # 256 features per core-headgroup
CH = 512           # token chunk (matmul moving free dim)
NCH = N // CH      # 4
KT = DIM // 128    # 8 k-tiles over model dim
TB = N // 128      # 16 token blocks
DVE_KBS = (1, 4, 7, 10, 13)  # kb slabs per unit whose exp runs on DVE (fast-exp)
K_SHIFT = 2.5      # global score shift: E = exp(s - K), cancels in softmax,
                   # keeps exp values in fp8-e4m3 range (row maxes are ~0.8-3.1)
EXP8_A = SCALE * 8.0 / float(np.log(2.0))          # schraudolph->e4m3 multiplier
EXP8_B = 7.0 * 8.0 - 0.44 - K_SHIFT * 8.0 / float(np.log(2.0))
_cache = {}


def _build():
    from contextlib import ExitStack

    import concourse.mybir as mybir
    from concourse import bacc
    from concourse.tile import TileContext

    f32 = mybir.dt.float32
    bf16 = mybir.dt.bfloat16
    f8 = mybir.dt.float8e4
    u8 = mybir.dt.uint8
    nc = bacc.Bacc("TRN2", target_bir_lowering=False)

    xt_d = nc.declare_dram_parameter("xt", [DIM, N], bf16, isOutput=False)
    wqk_d = nc.declare_dram_parameter("wqk", [DIM, 2 * F], bf16, isOutput=False)
    wv_d = nc.declare_dram_parameter("wv", [DIM, F], bf16, isOutput=False)
    bqk_d = nc.declare_dram_parameter("bqk", [2 * F], f32, isOutput=False)
    bv_d = nc.declare_dram_parameter("bv", [F], f32, isOutput=False)
    pw_d = nc.declare_dram_parameter("pw", [F, DIM], bf16, isOutput=False)
    out_d = nc.declare_dram_parameter("out", [DIM, N], bf16, isOutput=True)
    rscr = nc.dram_tensor("rscr", [2, NCH, 2 * CH], f32)

    # chunk-major views: one DMA delivers [128, 8, *] (all 8 k-tiles)
    xt_r = xt_d.ap().rearrange("(t p) n -> p t n", p=128)
    wqk_r = wqk_d.ap().rearrange("(t p) m -> p t m", p=128)
    wv_r = wv_d.ap().rearrange("(t p) m -> p t m", p=128)
    pw_r = pw_d.ap().rearrange("(t p) m -> t p m", p=128)
    out_r = out_d.ap().rearrange("(t p) n -> t p n", p=128)

    with TileContext(nc) as tc, ExitStack() as st:
        consts = st.enter_context(tc.tile_pool(name="consts", bufs=1))
        qkp = st.enter_context(tc.tile_pool(name="qkp", bufs=1))
        vtp = st.enter_context(tc.tile_pool(name="vtp", bufs=1))
        otp = st.enter_context(tc.tile_pool(name="otp", bufs=1))
        ep = st.enter_context(tc.tile_pool(name="ep", bufs=2))
        recp = st.enter_context(tc.tile_pool(name="recp", bufs=2))
        outs = st.enter_context(tc.tile_pool(name="outs", bufs=3))
        stgp = st.enter_context(tc.tile_pool(name="stgp", bufs=2))
        xw = st.enter_context(tc.tile_pool(name="xw", bufs=1))
        ps_mm = st.enter_context(tc.tile_pool(name="ps_mm", bufs=2, space="PSUM"))
        ps_s = st.enter_context(tc.tile_pool(name="ps_s", bufs=2, space="PSUM"))
        ps_av = st.enter_context(tc.tile_pool(name="ps_av", bufs=2, space="PSUM"))

        # ---- constant + weight tiles -------------------------------------
        bqk_sb = consts.tile([128, 2 * F // 128], f32)
        bv_sb = consts.tile([128, F], f32)
        nks_sb = consts.tile([128, 1], f32)
        nc.vector.memset(nks_sb, -K_SHIFT)
        # host supplies wqk with columns reordered to [k01 | q01 | k23 | q23]
        wkq01_sb = xw.tile([128, KT, F], bf16)
        wkq23_sb = xw.tile([128, KT, F], bf16)
        wv_sb = xw.tile([128, KT, F], bf16)
        pw_sb = [consts.tile([128, DIM], bf16, tag=f"pw{t}", name=f"pw{t}")
                 for t in range(2)]
        xt_sb = [xw.tile([128, KT, CH], bf16, tag=f"x{ch}", name=f"x{ch}")
                 for ch in range(NCH)]

        # DMA plan (issue ~0.6us fixed, ~150GB/s per ring, keep elements
        # >=512B).  Ring loads: sync: kq01, x0b, x1 | scalar: wv, kq23,
        # x3, pw | gpsimd: x0a, biases, x2.  k01+q01+x0 gate the first
        # score matmul.
        nc.sync.dma_start(out=wkq01_sb[:, :4, :], in_=wqk_r[:, :4, :F])
        nc.gpsimd.dma_start(out=xt_sb[0][:, :4, :], in_=xt_r[:, :4, 0 * CH:1 * CH])
        nc.scalar.dma_start(out=wv_sb, in_=wv_r)
        nc.sync.dma_start(out=wkq01_sb[:, 4:, :], in_=wqk_r[:, 4:, :F])
        nc.gpsimd.dma_start(out=bqk_sb, in_=bqk_d.ap().rearrange("(f p) -> p f", p=128))
        nc.gpsimd.dma_start(out=bv_sb, in_=bv_d.ap().partition_broadcast(128))
        nc.gpsimd.dma_start(out=xt_sb[0][:, 4:, :], in_=xt_r[:, 4:, 0 * CH:1 * CH])
        nc.scalar.dma_start(out=wkq23_sb, in_=wqk_r[:, :, F:])
        nc.sync.dma_start(out=xt_sb[1], in_=xt_r[:, :, 1 * CH:2 * CH])
        nc.gpsimd.dma_start(out=xt_sb[2], in_=xt_r[:, :, 2 * CH:3 * CH])
        nc.sync.dma_start(out=xt_sb[3], in_=xt_r[:, :, 3 * CH:4 * CH])
        for t in range(2):
            nc.gpsimd.dma_start(out=pw_sb[t], in_=pw_r[t])

        # ---- working tiles ----------------------------------------------
        qk_sb = [[qkp.tile([128, CH], bf16, tag=f"qk{fb}_{ch}", name=f"qk{fb}_{ch}")
                  for ch in range(NCH)] for fb in range(4)]
        # fp8 V, two token-blocks per tile (the DoubleRow k-subtile pair)
        vt_sb = [vtp.tile([128, 2, HPC, DH + 4], f8, tag=f"vt{tb2}", name=f"vt{tb2}")
                 for tb2 in range(TB // 2)]
        ot_sb = [[otp.tile([128, CH], bf16, tag=f"ot{t}_{ch}", name=f"ot{t}_{ch}")
                  for ch in range(NCH)] for t in range(2)]

        def emit_qk_group(fb, ch, pool, c0=0, c1=CH):
            # fb: 0=q01 1=q23 2=k01 3=k23; host column order [k01 q01 k23 q23]
            w = wkq01_sb if fb % 2 == 0 else wkq23_sb
            wo = 0 if fb >= 2 else 128
            ps = pool.tile([128, c1 - c0], f32,
                           tag=pool.name.startswith("ps_s") and "sp" or "mm",
                           name=f"qkg{fb}_{ch}")
            for t in range(KT):
                nc.tensor.matmul(
                    ps,
                    w[:, t, wo:wo + 128],
                    xt_sb[ch][:, t, c0:c1],
                    start=(t == 0), stop=(t == KT - 1),
                )
            nc.scalar.activation(
                out=qk_sb[fb][ch][:, c0:c1], in_=ps,
                func=mybir.ActivationFunctionType.Identity,
                bias=bqk_sb[:, fb:fb + 1],
            )

        def emit_v_group(tb, pool=None):
            # prologue: borrows the (still unused) AV psum slots; in-unit
            # emissions use the mm staging slots instead
            pool = pool or ps_av
            ps = pool.tile([128, F], f32,
                           tag="av" if pool is ps_av else "mm", name=f"vg{tb}")
            ch, blk = tb // 4, tb % 4
            for t in range(KT):
                nc.tensor.matmul(
                    ps,
                    xt_sb[ch][:, t, blk * 128:(blk + 1) * 128],
                    wv_sb[:, t, :],
                    start=(t == 0), stop=(t == KT - 1),
                )
            nc.vector.tensor_add(
                out=vt_sb[tb // 2][:, tb % 2, :, :DH],
                in0=ps.rearrange("p (h d) -> p h d", h=HPC),
                in1=bv_sb.rearrange("p (h d) -> p h d", h=HPC),
            )
            nc.vector.memset(vt_sb[tb // 2][:, tb % 2, :, DH:], 0.0)
            nc.vector.memset(vt_sb[tb // 2][:, tb % 2, :, DH:DH + 1], 1.0)

        # ---- attention units: unit = (head-pair hp, chunk cc) ------------
        units = [(hp, cc) for cc in range(NCH) for hp in (0, 1)]
        et_store = {}

        def q_slice(h, cc):
            return qk_sb[h // 2][cc][(h % 2) * 64:(h % 2) * 64 + 64, :]

        def k_slice(h, kb):
            t = qk_sb[2 + h // 2][kb // 4]
            return t[(h % 2) * 64:(h % 2) * 64 + 64, (kb % 4) * 128:(kb % 4 + 1) * 128]

        def emit_s(u, kb):
            # scores for slab kb; exp -> fp8 half of the paired E tile
            hp, cc = u
            sp = ps_s.tile([128, 2 * CH], f32, tag="sp", name=f"sp{hp}_{cc}_{kb}")
            for j in range(2):
                h = 2 * hp + j
                nc.tensor.matmul(
                    sp[:, j * CH:(j + 1) * CH],
                    k_slice(h, kb),
                    q_slice(h, cc),
                    start=True, stop=True,
                )
            kb2 = kb // 2
            if kb % 2 == 0:
                e = ep.tile([128, 2, 2 * CH], f8, tag=f"e{kb2}",
                            name=f"e{hp}_{cc}_{kb2}")
                et_store[u][kb2] = e
            else:
                e = et_store[u][kb2]
            eh = e[:, kb % 2, :]
            if kb in DVE_KBS:
                nc.vector.tensor_scalar(
                    out=eh.bitcast(u8), in0=sp,
                    scalar1=float(EXP8_A), scalar2=float(EXP8_B),
                    op0=mybir.AluOpType.mult, op1=mybir.AluOpType.add,
                )
            else:
                nc.scalar.activation(
                    out=eh, in_=sp,
                    func=mybir.ActivationFunctionType.Exp, scale=SCALE,
                    bias=nks_sb[:, 0:1],
                )
            et_store[u][kb2] = e

        dma_rr = [nc.sync, nc.gpsimd, nc.scalar]

        def proj_ops(fb, cc):
            # projection group split into two single-matmul filler ops; the
            # psum drain runs on the (slack) ACT engine
            state = {}

            def op1():
                ps = ps_mm.tile([128, CH], f32, tag="mm", name=f"pj{fb}_{cc}")
                state["ps"] = ps
                nc.tensor.matmul(ps, pw_sb[0][:, fb * 128:(fb + 1) * 128],
                                 ot_sb[0][cc], start=True, stop=False)

            def op2():
                ps = state["ps"]
                nc.tensor.matmul(ps, pw_sb[1][:, fb * 128:(fb + 1) * 128],
                                 ot_sb[1][cc], start=False, stop=True)
                os = outs.tile([128, CH], bf16, tag="os", name=f"os{fb}_{cc}")
                if fb % 2:
                    nc.scalar.copy(out=os, in_=ps)
                else:
                    nc.vector.tensor_copy(out=os, in_=ps)
                dma_rr[fb % 3].dma_start(
                    out=out_r[fb][:, cc * CH:(cc + 1) * CH], in_=os
                )

            return [("proj", op1), ("proj", op2)]

        def qk_ops(fb, ch):
            # QK group as 8 single-matmul ops + bias drain on the last
            w = wkq01_sb if fb % 2 == 0 else wkq23_sb
            wo = 0 if fb >= 2 else 128
            state = {}
            ops = []

            def mk(t):
                def op():
                    if t == 0:
                        state["ps"] = ps_mm.tile([128, CH], f32, tag="mm",
                                                 name=f"qkg{fb}_{ch}")
                    nc.tensor.matmul(state["ps"], w[:, t, wo:wo + 128],
                                     xt_sb[ch][:, t, :],
                                     start=(t == 0), stop=(t == KT - 1))
                    if t == KT - 1:
                        nc.scalar.activation(
                            out=qk_sb[fb][ch], in_=state["ps"],
                            func=mybir.ActivationFunctionType.Identity,
                            bias=bqk_sb[:, fb:fb + 1])
                return op

            for t in range(KT):
                ops.append(("qk", mk(t)))
            return ops

        # ---- prologue ----------------------------------------------------
        # q01(c0) full + only kb0's k01 columns first so the score/exp
        # stream starts as early as possible; the two groups interleave in
        # t-halves to cover the x0 second-half DMA gap.  k01's remaining
        # columns follow right behind S(kb0).
        psq = ps_s.tile([128, CH], f32, tag="sp", name="q01g")
        psk = ps_s.tile([128, 128], f32, tag="sp", name="k01p1")
        for t in range(4):
            nc.tensor.matmul(psq, wkq01_sb[:, t, 128:], xt_sb[0][:, t, :],
                             start=(t == 0), stop=False)
        for t in range(4):
            nc.tensor.matmul(psk, wkq01_sb[:, t, :128], xt_sb[0][:, t, :128],
                             start=(t == 0), stop=False)
        for t in range(4, KT):
            nc.tensor.matmul(psq, wkq01_sb[:, t, 128:], xt_sb[0][:, t, :],
                             start=False, stop=(t == KT - 1))
        nc.scalar.activation(out=qk_sb[0][0], in_=psq,
                             func=mybir.ActivationFunctionType.Identity,
                             bias=bqk_sb[:, 0:1])
        for t in range(4, KT):
            nc.tensor.matmul(psk, wkq01_sb[:, t, :128], xt_sb[0][:, t, :128],
                             start=False, stop=(t == KT - 1))
        nc.scalar.activation(out=qk_sb[2][0][:, :128], in_=psk,
                             func=mybir.ActivationFunctionType.Identity,
                             bias=bqk_sb[:, 2:3])
        et_store[units[0]] = [None] * (TB // 2)
        # per-kb filler: one V group per slab; k01(c+1) before S needs it;
        # k23/q23 chunk0 early so unit (1,0)'s score stream can start.
        pro_fill = {0: [(2, 0, ps_mm, 128, CH)],
                    1: [(3, 0, ps_mm, 0, CH)], 3: [(2, 1, ps_mm, 0, CH)],
                    5: [(1, 0, ps_mm, 0, CH)], 7: [(2, 2, ps_mm, 0, CH)],
                    11: [(2, 3, ps_mm, 0, CH)]}
        for kb in range(TB):
            emit_s(units[0], kb)
            if kb < 8:
                emit_v_group(kb)
            elif kb % 2 == 0:
                emit_v_group(8 + (kb - 8) // 2)
            for fb, ch, pool, c0, c1 in pro_fill.get(kb, ()):
                emit_qk_group(fb, ch, pool, c0, c1)

        # filler op queues per unit: QK groups feed the stream two units
        # ahead; proj drains chunks closed by the preceding (1,*) unit.
        os3 = []
        def v_op(tb):
            return ("v", lambda: emit_v_group(tb, ps_mm))

        fillq = {
            0: [v_op(12), v_op(13), v_op(14), v_op(15)]
               + qk_ops(3, 1) + qk_ops(3, 2) + qk_ops(3, 3) + qk_ops(0, 1),
            1: qk_ops(1, 1),
            2: qk_ops(0, 2) + sum((proj_ops(fb, 0) for fb in range(KT)), []),
            3: qk_ops(1, 2),
            4: qk_ops(0, 3) + sum((proj_ops(fb, 1) for fb in range(KT)), []),
            5: qk_ops(1, 3),
            6: (proj_ops(0, 2) + proj_ops(1, 2) + proj_ops(2, 2)
                + proj_ops(3, 2) + proj_ops(4, 2) + proj_ops(5, 2)
                + proj_ops(6, 2) + proj_ops(7, 2)),
        }
        os3 = []

        # chunk-3 projection prefill: t=0 halves accumulate into open psum
        # groups parked in the freed mm/score banks; the tail then just adds
        # the t=1 half and drains once on the (idle) ACT engine.
        os3ps = {}

        def os3_prefill(fb):
            def op():
                if fb < 2:
                    ps = ps_mm.tile([128, CH], f32, tag="mm", name=f"pj3a{fb}")
                else:
                    sl = os3ps.get(("sp", (fb - 2) // 2))
                    if sl is None:
                        sl = ps_s.tile([128, 2 * CH], f32, tag="sp",
                                       name=f"pj3sp{(fb - 2) // 2}")
                        os3ps[("sp", (fb - 2) // 2)] = sl
                    ps = sl[:, (fb % 2) * CH:(fb % 2 + 1) * CH]
                os3ps[fb] = ps
                nc.tensor.matmul(ps, pw_sb[0][:, fb * 128:(fb + 1) * 128],
                                 ot_sb[0][NCH - 1], start=True, stop=False)
            return ("proj", op)

        fillq[7] = [os3_prefill(fb) for fb in range(6)]

        for i, u in enumerate(units):
            hp, cc = u
            nxt = units[i + 1] if i + 1 < len(units) else None
            if nxt is not None:
                et_store[nxt] = [None] * (TB // 2)
            avs = [
                ps_av.tile([68, CH], f32, tag="av", name=f"av{hp}_{cc}_{j}")
                for j in range(2)
            ]
            ops = fillq.get(i, [])
            for kb in range(TB):
                if nxt is not None:
                    emit_s(nxt, kb)
                # one fp8 DoubleRow AV matmul per slot (2 k-subtiles each):
                # head j = kb%2 over token-block pair kb2 = kb//2
                j, kb2 = kb % 2, kb // 2
                nc.tensor.matmul(
                    avs[j],
                    vt_sb[kb2][:, :, 2 * hp + j, :],
                    et_store[u][kb2][:, :, j * CH:(j + 1) * CH],
                    start=(kb2 == 0), stop=(kb2 == TB // 2 - 1),
                    perf_mode=mybir.MatmulPerfMode.DoubleRow,
                )
                # pop fillers: enough each slot to drain the queue by unit end
                npop = -(-len(ops) // (TB - kb))
                for _ in range(min(npop, 3)):
                    if ops:
                        ops.pop(0)[1]()
            for kind, op in ops:
                op()
            et_store.pop(u)

            # epilogue: drain AV psum to SBUF (frees banks), reciprocal of
            # the ones-row, broadcast via DRAM round-trip, normalize on Pool
            stg = stgp.tile([65, 2 * CH], f32, tag="stg", name=f"stg{hp}_{cc}")
            sums = recp.tile([1, 2 * CH], f32, tag="sums", name=f"sums{hp}_{cc}")
            nc.vector.tensor_copy(out=stg[:, 0 * CH:1 * CH], in_=avs[0][:65, :])
            nc.scalar.copy(out=stg[:, 1 * CH:2 * CH], in_=avs[1][:65, :])
            nc.vector.tensor_copy(out=sums[:, 0 * CH:1 * CH], in_=avs[0][64:65, :])
            nc.vector.tensor_copy(out=sums[:, 1 * CH:2 * CH], in_=avs[1][64:65, :])
            rec = recp.tile([1, 2 * CH], f32, tag="rec", name=f"rec{hp}_{cc}")
            nc.vector.reciprocal_approx_fast(out=rec, in_=sums)
            nc.gpsimd.dma_start(out=rscr.ap()[hp, cc], in_=rec)
            rec64 = recp.tile([64, 2 * CH], f32, tag="rec64", name=f"rb{hp}_{cc}")
            nc.gpsimd.dma_start(
                out=rec64, in_=rscr.ap()[hp, cc].partition_broadcast(64)
            )
            for j, eng in ((0, nc.gpsimd), (1, nc.vector)):
                h = 2 * hp + j
                eng.tensor_mul(
                    out=ot_sb[h // 2][cc][(h % 2) * 64:(h % 2) * 64 + 64, :],
                    in0=stg[0:64, j * CH:(j + 1) * CH],
                    in1=rec64[:, j * CH:(j + 1) * CH],
                )
        # chunk-3 projection tail: close the prefilled groups, drain on ACT
        def os3_drain(fb):
            os = outs.tile([128, CH], bf16, tag=f"os3_{fb}", bufs=1,
                           name=f"os3_{fb}")
            nc.scalar.copy(out=os, in_=os3ps[fb])
            dma_rr[fb % 3].dma_start(
                out=out_r[fb][:, (NCH - 1) * CH:NCH * CH], in_=os
            )

        for fb in range(6):
            nc.tensor.matmul(os3ps[fb], pw_sb[1][:, fb * 128:(fb + 1) * 128],
                             ot_sb[1][NCH - 1], start=False, stop=True)
        os3_drain(0)
        os3_drain(1)
        for fb in range(6, KT):
            ps = ps_mm.tile([128, CH], f32, tag="mm", name=f"pj3b{fb}")
            os3ps[fb] = ps
            for t in range(2):
                nc.tensor.matmul(ps, pw_sb[t][:, fb * 128:(fb + 1) * 128],
                                 ot_sb[t][NCH - 1], start=(t == 0), stop=(t == 1))
        for fb in range(2, KT):
            os3_drain(fb)

    nc.finalize()
    return nc


def _in_maps(x, qkv_w, qkv_b, proj_w):
    import ml_dtypes

    bf = ml_dtypes.bfloat16
    maps = []
    for c in range(NCORE):
        b, hg = c // 4, c % 4
        fs = slice(hg * F, (hg + 1) * F)
        q, k = qkv_w[fs], qkv_w[DIM:][fs]
        # device column order [k01 | q01 | k23 | q23]
        wqk = np.concatenate([k[:128], q[:128], k[128:], q[128:]], 0)  # [512,1024]
        bqk = np.concatenate([qkv_b[fs], qkv_b[DIM:][fs]], 0)
        maps.append({
            "xt": np.ascontiguousarray(x[b].T).astype(bf),
            "wqk": np.ascontiguousarray(wqk.T).astype(bf),
            "wv": np.ascontiguousarray(qkv_w[2 * DIM:][fs].T).astype(bf),
            "bqk": np.ascontiguousarray(bqk),
            "bv": np.ascontiguousarray(qkv_b[2 * DIM:][fs]),
            "pw": np.ascontiguousarray(proj_w[:, fs].T).astype(bf),
        })
    return maps


def _run(inputs, trace=False, trace_kwargs=None):
    from concourse.bass_utils import run_bass_kernel_spmd

    if "nc" not in _cache:
        _cache["nc"] = _build()
    nc = _cache["nc"]
    maps = _in_maps(inputs["x"], inputs["qkv_w"], inputs["qkv_b"], inputs["proj_w"])
    res = run_bass_kernel_spmd(
        nc, maps, list(range(NCORE)), trace=trace, **(trace_kwargs or {})
    )
    outs = [r["out"] for r in res.results]              # [1024, 2048] bf16 partials
    full = np.empty((B, N, DIM), dtype=np.float32)
    for b in range(B):
        acc = outs[4 * b].astype(np.float32)
        for c in range(4 * b + 1, 4 * b + 4):
            acc += outs[c].astype(np.float32)
        full[b] = acc.T + inputs["proj_b"]
    return full, res


def kernel(**inputs) -> np.ndarray:
    out, _ = _run(inputs, trace=False)
    return out


# revision 52
# speedup vs baseline: 1.2011x; 1.0056x over previous
"""Multi-head attention (B=2, N=2048, D=1024, H=16) on 8 TRN2 NeuronCores.

Sharding: core c handles batch b=c//4 and head group hg=c%4 (4 heads of 16).
Each core computes QKV for its heads, materialized attention, and a partial
projection (proj row-split over heads); the host sums 4 partials per batch
and adds proj bias.  No device collectives.

v2 schedule, engineered to the PE roofline (~136.5us of moving-row time):
  - chunk-granular input DMA (one descriptor-batch per x chunk) spread over
    4 queues so the first score matmul fires at ~6us
  - hp-interleaved unit order (0,0),(1,0),(0,1),(1,1),... so projection
    work for chunk cc unlocks right after unit (1,cc) and spreads forward
  - exp split: most kb-slabs on ACT (hardware Exp), kbs in DVE_KBS per unit
    computed on the Vector engine with a Schraudolph fast-exp (scores*A+B
    -> int16 -> bitcast bf16), keeping ACT under the PE floor
  - V bias-add + normalize-mul + proj-psum drain on Pool, score bias +
    AV-psum drain + reciprocal on Vector: no engine above ~60% of the span
  - PSUM: 4 banks score double-buffer, 2 banks AV accumulators, 2 banks
    shared QKV/V/proj staging (prologue QK groups borrow the score banks)
"""

import numpy as np

B, N, DIM, H, DH = 2, 2048, 1024, 16, 64
SCALE = DH ** -0.5
NCORE = 8
HPC = 4            # heads per core
F = HPC * DH       # 256 features per core-headgroup
CH = 512           # token chunk (matmul moving free dim)
NCH = N // CH      # 4
KT = DIM // 128    # 8 k-tiles over model dim
TB = N // 128      # 16 token blocks
DVE_KBS = (1, 4, 7, 10, 13)  # kb slabs per unit whose exp runs on DVE (fast-exp)
K_SHIFT = 2.5      # global score shift: E = exp(s - K), cancels in softmax,
                   # keeps exp values in fp8-e4m3 range (row maxes are ~0.8-3.1)
EXP8_A = SCALE * 8.0 / float(np.log(2.0))          # schraudolph->e4m3 multiplier
EXP8_B = 7.0 * 8.0 - 0.44 - K_SHIFT * 8.0 / float(np.log(2.0))
_cache = {}


def _build():
    from contextlib import ExitStack

    import concourse.mybir as mybir
    from concourse import bacc
    from concourse.tile import TileContext

    f32 = mybir.dt.float32
    bf16 = mybir.dt.bfloat16
    f8 = mybir.dt.float8e4
    u8 = mybir.dt.uint8
    nc = bacc.Bacc("TRN2", target_bir_lowering=False)

    xt_d = nc.declare_dram_parameter("xt", [DIM, N], bf16, isOutput=False)
    wqk_d = nc.declare_dram_parameter("wqk", [DIM, 2 * F], bf16, isOutput=False)
    wv_d = nc.declare_dram_parameter("wv", [DIM, F], bf16, isOutput=False)
    bqk_d = nc.declare_dram_parameter("bqk", [2 * F], f32, isOutput=False)
    bv_d = nc.declare_dram_parameter("bv", [F], f32, isOutput=False)
    pw_d = nc.declare_dram_parameter("pw", [F, DIM], bf16, isOutput=False)
    out_d = nc.declare_dram_parameter("out", [DIM, N], bf16, isOutput=True)
    rscr = nc.dram_tensor("rscr", [2, NCH, 2 * CH], f32)

    # chunk-major views: one DMA delivers [128, 8, *] (all 8 k-tiles)
    xt_r = xt_d.ap().rearrange("(t p) n -> p t n", p=128)
    wqk_r = wqk_d.ap().rearrange("(t p) m -> p t m", p=128)
    wv_r = wv_d.ap().rearrange("(t p) m -> p t m", p=128)
    pw_r = pw_d.ap().rearrange("(t p) m -> t p m", p=128)
    out_r = out_d.ap().rearrange("(t p) n -> t p n", p=128)

    with TileContext(nc) as tc, ExitStack() as st:
        consts = st.enter_context(tc.tile_pool(name="consts", bufs=1))
        qkp = st.enter_context(tc.tile_pool(name="qkp", bufs=1))
        vtp = st.enter_context(tc.tile_pool(name="vtp", bufs=1))
        otp = st.enter_context(tc.tile_pool(name="otp", bufs=1))
        ep = st.enter_context(tc.tile_pool(name="ep", bufs=2))
        recp = st.enter_context(tc.tile_pool(name="recp", bufs=2))
        outs = st.enter_context(tc.tile_pool(name="outs", bufs=3))
        stgp = st.enter_context(tc.tile_pool(name="stgp", bufs=2))
        xw = st.enter_context(tc.tile_pool(name="xw", bufs=1))
        ps_mm = st.enter_context(tc.tile_pool(name="ps_mm", bufs=2, space="PSUM"))
        ps_s = st.enter_context(tc.tile_pool(name="ps_s", bufs=2, space="PSUM"))
        ps_av = st.enter_context(tc.tile_pool(name="ps_av", bufs=2, space="PSUM"))

        # ---- constant + weight tiles -------------------------------------
        bqk_sb = consts.tile([128, 2 * F // 128], f32)
        bv_sb = consts.tile([128, F], f32)
        nks_sb = consts.tile([128, 1], f32)
        nc.vector.memset(nks_sb, -K_SHIFT)
        # host supplies wqk with columns reordered to [k01 | q01 | k23 | q23]
        wkq01_sb = xw.tile([128, KT, F], bf16)
        wkq23_sb = xw.tile([128, KT, F], bf16)
        wv_sb = xw.tile([128, KT, F], bf16)
        pw_sb = [consts.tile([128, DIM], bf16, tag=f"pw{t}", name=f"pw{t}")
                 for t in range(2)]
        xt_sb = [xw.tile([128, KT, CH], bf16, tag=f"x{ch}", name=f"x{ch}")
                 for ch in range(NCH)]

        # DMA plan (issue ~0.6us fixed, ~150GB/s per ring, keep elements
        # >=512B).  Ring loads: sync: kq01, x0b, x1 | scalar: wv, kq23,
        # x3, pw | gpsimd: x0a, biases, x2.  k01+q01+x0 gate the first
        # score matmul.
        nc.sync.dma_start(out=wkq01_sb[:, :4, :], in_=wqk_r[:, :4, :F])
        nc.gpsimd.dma_start(out=xt_sb[0][:, :4, :], in_=xt_r[:, :4, 0 * CH:1 * CH])
        nc.scalar.dma_start(out=wv_sb, in_=wv_r)
        nc.sync.dma_start(out=wkq01_sb[:, 4:, :], in_=wqk_r[:, 4:, :F])
        nc.gpsimd.dma_start(out=bqk_sb, in_=bqk_d.ap().rearrange("(f p) -> p f", p=128))
        nc.gpsimd.dma_start(out=bv_sb, in_=bv_d.ap().partition_broadcast(128))
        nc.gpsimd.dma_start(out=xt_sb[0][:, 4:, :], in_=xt_r[:, 4:, 0 * CH:1 * CH])
        nc.scalar.dma_start(out=wkq23_sb, in_=wqk_r[:, :, F:])
        nc.sync.dma_start(out=xt_sb[1], in_=xt_r[:, :, 1 * CH:2 * CH])
        nc.gpsimd.dma_start(out=xt_sb[2], in_=xt_r[:, :, 2 * CH:3 * CH])
        nc.sync.dma_start(out=xt_sb[3], in_=xt_r[:, :, 3 * CH:4 * CH])
        for t in range(2):
            nc.gpsimd.dma_start(out=pw_sb[t], in_=pw_r[t])

        # ---- working tiles ----------------------------------------------
        qk_sb = [[qkp.tile([128, CH], bf16, tag=f"qk{fb}_{ch}", name=f"qk{fb}_{ch}")
                  for ch in range(NCH)] for fb in range(4)]
        # fp8 V, two token-blocks per tile (the DoubleRow k-subtile pair)
        vt_sb = [vtp.tile([128, 2, HPC, DH + 4], f8, tag=f"vt{tb2}", name=f"vt{tb2}")
                 for tb2 in range(TB // 2)]
        ot_sb = [[otp.tile([128, CH], bf16, tag=f"ot{t}_{ch}", name=f"ot{t}_{ch}")
                  for ch in range(NCH)] for t in range(2)]

        def emit_qk_group(fb, ch, pool, c0=0, c1=CH):
            # fb: 0=q01 1=q23 2=k01 3=k23; host column order [k01 q01 k23 q23]
            w = wkq01_sb if fb % 2 == 0 else wkq23_sb
            wo = 0 if fb >= 2 else 128
            ps = pool.tile([128, c1 - c0], f32,
                           tag=pool.name.startswith("ps_s") and "sp" or "mm",
                           name=f"qkg{fb}_{ch}")
            for t in range(KT):
                nc.tensor.matmul(
                    ps,
                    w[:, t, wo:wo + 128],
                    xt_sb[ch][:, t, c0:c1],
                    start=(t == 0), stop=(t == KT - 1),
                )
            nc.scalar.activation(
                out=qk_sb[fb][ch][:, c0:c1], in_=ps,
                func=mybir.ActivationFunctionType.Identity,
                bias=bqk_sb[:, fb:fb + 1],
            )

        def emit_v_group(tb, pool=None):
            # prologue: borrows the (still unused) AV psum slots; in-unit
            # emissions use the mm staging slots instead
            pool = pool or ps_av
            ps = pool.tile([128, F], f32,
                           tag="av" if pool is ps_av else "mm", name=f"vg{tb}")
            ch, blk = tb // 4, tb % 4
            for t in range(KT):
                nc.tensor.matmul(
                    ps,
                    xt_sb[ch][:, t, blk * 128:(blk + 1) * 128],
                    wv_sb[:, t, :],
                    start=(t == 0), stop=(t == KT - 1),
                )
            nc.vector.tensor_add(
                out=vt_sb[tb // 2][:, tb % 2, :, :DH],
                in0=ps.rearrange("p (h d) -> p h d", h=HPC),
                in1=bv_sb.rearrange("p (h d) -> p h d", h=HPC),
            )
            nc.vector.memset(vt_sb[tb // 2][:, tb % 2, :, DH:], 0.0)
            nc.vector.memset(vt_sb[tb // 2][:, tb % 2, :, DH:DH + 1], 1.0)

        # ---- attention units: unit = (head-pair hp, chunk cc) ------------
        units = [(hp, cc) for cc in range(NCH) for hp in (0, 1)]
        et_store = {}

        def q_slice(h, cc):
            return qk_sb[h // 2][cc][(h % 2) * 64:(h % 2) * 64 + 64, :]

        def k_slice(h, kb):
            t = qk_sb[2 + h // 2][kb // 4]
            return t[(h % 2) * 64:(h % 2) * 64 + 64, (kb % 4) * 128:(kb % 4 + 1) * 128]

        def emit_s(u, kb):
            # scores for slab kb; exp -> fp8 half of the paired E tile
            hp, cc = u
            sp = ps_s.tile([128, 2 * CH], f32, tag="sp", name=f"sp{hp}_{cc}_{kb}")
            for j in range(2):
                h = 2 * hp + j
                nc.tensor.matmul(
                    sp[:, j * CH:(j + 1) * CH],
                    k_slice(h, kb),
                    q_slice(h, cc),
                    start=True, stop=True,
                )
            kb2 = kb // 2
            if kb % 2 == 0:
                e = ep.tile([128, 2, 2 * CH], f8, tag=f"e{kb2}",
                            name=f"e{hp}_{cc}_{kb2}")
                et_store[u][kb2] = e
            else:
                e = et_store[u][kb2]
            eh = e[:, kb % 2, :]
            if kb in DVE_KBS:
                nc.vector.tensor_scalar(
                    out=eh.bitcast(u8), in0=sp,
                    scalar1=float(EXP8_A), scalar2=float(EXP8_B),
                    op0=mybir.AluOpType.mult, op1=mybir.AluOpType.add,
                )
            else:
                nc.scalar.activation(
                    out=eh, in_=sp,
                    func=mybir.ActivationFunctionType.Exp, scale=SCALE,
                    bias=nks_sb[:, 0:1],
                )
            et_store[u][kb2] = e

        dma_rr = [nc.sync, nc.scalar, nc.sync]

        def proj_ops(fb, cc):
            # projection group split into two single-matmul filler ops; the
            # psum drain runs on the (slack) ACT engine
            state = {}

            def op1():
                ps = ps_mm.tile([128, CH], f32, tag="mm", name=f"pj{fb}_{cc}")
                state["ps"] = ps
                nc.tensor.matmul(ps, pw_sb[0][:, fb * 128:(fb + 1) * 128],
                                 ot_sb[0][cc], start=True, stop=False)

            def op2():
                ps = state["ps"]
                nc.tensor.matmul(ps, pw_sb[1][:, fb * 128:(fb + 1) * 128],
                                 ot_sb[1][cc], start=False, stop=True)
                os = outs.tile([128, CH], bf16, tag="os", name=f"os{fb}_{cc}")
                if fb % 2:
                    nc.scalar.copy(out=os, in_=ps)
                else:
                    nc.vector.tensor_copy(out=os, in_=ps)
                dma_rr[fb % 3].dma_start(
                    out=out_r[fb][:, cc * CH:(cc + 1) * CH], in_=os
                )

            return [("proj", op1), ("proj", op2)]

        def qk_ops(fb, ch):
            # QK group as 8 single-matmul ops + bias drain on the last
            w = wkq01_sb if fb % 2 == 0 else wkq23_sb
            wo = 0 if fb >= 2 else 128
            state = {}
            ops = []

            def mk(t):
                def op():
                    if t == 0:
                        state["ps"] = ps_mm.tile([128, CH], f32, tag="mm",
                                                 name=f"qkg{fb}_{ch}")
                    nc.tensor.matmul(state["ps"], w[:, t, wo:wo + 128],
                                     xt_sb[ch][:, t, :],
                                     start=(t == 0), stop=(t == KT - 1))
                    if t == KT - 1:
                        nc.scalar.activation(
                            out=qk_sb[fb][ch], in_=state["ps"],
                            func=mybir.ActivationFunctionType.Identity,
                            bias=bqk_sb[:, fb:fb + 1])
                return op

            for t in range(KT):
                ops.append(("qk", mk(t)))
            return ops

        # ---- prologue ----------------------------------------------------
        # q01(c0) full + only kb0's k01 columns first so the score/exp
        # stream starts as early as possible; the two groups interleave in
        # t-halves to cover the x0 second-half DMA gap.  k01's remaining
        # columns follow right behind S(kb0).
        psq = ps_s.tile([128, CH], f32, tag="sp", name="q01g")
        psk = ps_s.tile([128, 128], f32, tag="sp", name="k01p1")
        for t in range(4):
            nc.tensor.matmul(psq, wkq01_sb[:, t, 128:], xt_sb[0][:, t, :],
                             start=(t == 0), stop=False)
        for t in range(4):
            nc.tensor.matmul(psk, wkq01_sb[:, t, :128], xt_sb[0][:, t, :128],
                             start=(t == 0), stop=False)
        for t in range(4, KT):
            nc.tensor.matmul(psq, wkq01_sb[:, t, 128:], xt_sb[0][:, t, :],
                             start=False, stop=(t == KT - 1))
        nc.scalar.activation(out=qk_sb[0][0], in_=psq,
                             func=mybir.ActivationFunctionType.Identity,
                             bias=bqk_sb[:, 0:1])
        for t in range(4, KT):
            nc.tensor.matmul(psk, wkq01_sb[:, t, :128], xt_sb[0][:, t, :128],
                             start=False, stop=(t == KT - 1))
        nc.scalar.activation(out=qk_sb[2][0][:, :128], in_=psk,
                             func=mybir.ActivationFunctionType.Identity,
                             bias=bqk_sb[:, 2:3])
        et_store[units[0]] = [None] * (TB // 2)
        # per-kb filler: one V group per slab; k01(c+1) before S needs it;
        # k23/q23 chunk0 early so unit (1,0)'s score stream can start.
        pro_fill = {0: [(2, 0, ps_mm, 128, CH)],
                    1: [(3, 0, ps_mm, 0, CH)], 3: [(2, 1, ps_mm, 0, CH)],
                    5: [(1, 0, ps_mm, 0, CH)], 7: [(2, 2, ps_mm, 0, CH)],
                    11: [(2, 3, ps_mm, 0, CH)]}
        for kb in range(TB):
            emit_s(units[0], kb)
            if kb < 8:
                emit_v_group(kb)
            elif kb % 2 == 0:
                emit_v_group(8 + (kb - 8) // 2)
            for fb, ch, pool, c0, c1 in pro_fill.get(kb, ()):
                emit_qk_group(fb, ch, pool, c0, c1)

        # filler op queues per unit: QK groups feed the stream two units
        # ahead; proj drains chunks closed by the preceding (1,*) unit.
        os3 = []
        def v_op(tb):
            return ("v", lambda: emit_v_group(tb, ps_mm))

        fillq = {
            0: [v_op(12), v_op(13), v_op(14), v_op(15)]
               + qk_ops(3, 1) + qk_ops(3, 2) + qk_ops(3, 3) + qk_ops(0, 1),
            1: qk_ops(1, 1),
            2: qk_ops(0, 2) + sum((proj_ops(fb, 0) for fb in range(KT)), []),
            3: qk_ops(1, 2),
            4: qk_ops(0, 3) + sum((proj_ops(fb, 1) for fb in range(KT)), []),
            5: qk_ops(1, 3),
            6: (proj_ops(0, 2) + proj_ops(1, 2) + proj_ops(2, 2)
                + proj_ops(3, 2) + proj_ops(4, 2) + proj_ops(5, 2)
                + proj_ops(6, 2) + proj_ops(7, 2)),
        }
        os3 = []

        # chunk-3 projection prefill: t=0 halves accumulate into open psum
        # groups parked in the freed mm/score banks; the tail then just adds
        # the t=1 half and drains once on the (idle) ACT engine.
        os3ps = {}

        def os3_prefill(fb):
            def op():
                if fb < 2:
                    ps = ps_mm.tile([128, CH], f32, tag="mm", name=f"pj3a{fb}")
                else:
                    sl = os3ps.get(("sp", (fb - 2) // 2))
                    if sl is None:
                        sl = ps_s.tile([128, 2 * CH], f32, tag="sp",
                                       name=f"pj3sp{(fb - 2) // 2}")
                        os3ps[("sp", (fb - 2) // 2)] = sl
                    ps = sl[:, (fb % 2) * CH:(fb % 2 + 1) * CH]
                os3ps[fb] = ps
                nc.tensor.matmul(ps, pw_sb[0][:, fb * 128:(fb + 1) * 128],
                                 ot_sb[0][NCH - 1], start=True, stop=False)
            return ("proj", op)

        fillq[7] = [os3_prefill(fb) for fb in range(6)]

        def os3_drain(fb):
            os = outs.tile([128, CH], bf16, tag=f"os3_{fb}", bufs=1,
                           name=f"os3_{fb}")
            nc.scalar.copy(out=os, in_=os3ps[fb])
            dma_rr[fb % 3].dma_start(
                out=out_r[fb][:, (NCH - 1) * CH:NCH * CH], in_=os
            )

        def emit_last_unit(u, avs, ops, av_mm):
            # last unit, pipelined per head: head-0 AV + its epilogue run
            # while head-1 streams, so the reciprocal DMA round-trip hides;
            # the chunk-3 projection t=1 contribution is added in K=64
            # per-head halves as each head's ot rows become ready.
            hp, cc = u
            stg = stgp.tile([65, 2 * CH], f32, tag="stg", name="stgL")
            sums = recp.tile([1, 2 * CH], f32, tag="sums", name="sumsL")
            rec = recp.tile([1, 2 * CH], f32, tag="rec", name="recL")
            rec64 = recp.tile([64, 2 * CH], f32, tag="rec64", name="rbL")

            def epi_head(j):
                sl = np.s_[:, j * CH:(j + 1) * CH]
                nc.vector.tensor_copy(out=stg[sl], in_=avs[j][:65, :])
                nc.vector.tensor_copy(out=sums[sl], in_=avs[j][64:65, :])
                nc.vector.reciprocal_approx_fast(out=rec[sl], in_=sums[sl])
                nc.gpsimd.dma_start(out=rscr.ap()[hp, cc][j * CH:(j + 1) * CH],
                                    in_=rec[sl])
                nc.gpsimd.dma_start(
                    out=rec64[sl],
                    in_=rscr.ap()[hp, cc][j * CH:(j + 1) * CH]
                    .partition_broadcast(64))
                h = 2 * hp + j
                (nc.gpsimd if j == 0 else nc.vector).tensor_mul(
                    out=ot_sb[h // 2][cc][(h % 2) * 64:(h % 2) * 64 + 64, :],
                    in0=stg[0:64, j * CH:(j + 1) * CH],
                    in1=rec64[:, j * CH:(j + 1) * CH],
                )

            for kb2 in range(TB // 2):
                av_mm(0, kb2)
                if ops:
                    ops.pop(0)[1]()
            epi_head(0)
            for kb2 in range(TB // 2):
                av_mm(1, kb2)
                if ops:
                    ops.pop(0)[1]()
            ot1 = ot_sb[1][NCH - 1]
            for fb in range(6):
                nc.tensor.matmul(os3ps[fb], pw_sb[1][0:64, fb * 128:(fb + 1) * 128],
                                 ot1[0:64, :], start=False, stop=False)
            epi_head(1)
            for fb in range(6):
                nc.tensor.matmul(os3ps[fb], pw_sb[1][64:128, fb * 128:(fb + 1) * 128],
                                 ot1[64:128, :], start=False, stop=True)
            os3_drain(0)
            os3_drain(1)
            for fb in range(6, KT):
                ps = ps_mm.tile([128, CH], f32, tag="mm", name=f"pj3b{fb}")
                os3ps[fb] = ps
                for t in range(2):
                    nc.tensor.matmul(ps, pw_sb[t][:, fb * 128:(fb + 1) * 128],
                                     ot_sb[t][NCH - 1],
                                     start=(t == 0), stop=(t == 1))
            for fb in range(2, KT):
                os3_drain(fb)

        for i, u in enumerate(units):
            hp, cc = u
            nxt = units[i + 1] if i + 1 < len(units) else None
            if nxt is not None:
                et_store[nxt] = [None] * (TB // 2)
            avs = [
                ps_av.tile([68, CH], f32, tag="av", name=f"av{hp}_{cc}_{j}")
                for j in range(2)
            ]
            ops = fillq.get(i, [])

            def av_mm(j, kb2):
                nc.tensor.matmul(
                    avs[j],
                    vt_sb[kb2][:, :, 2 * hp + j, :],
                    et_store[u][kb2][:, :, j * CH:(j + 1) * CH],
                    start=(kb2 == 0), stop=(kb2 == TB // 2 - 1),
                    perf_mode=mybir.MatmulPerfMode.DoubleRow,
                )

            if i == len(units) - 1:
                emit_last_unit(u, avs, ops, av_mm)
                et_store.pop(u)
                continue
            for kb in range(TB):
                if nxt is not None:
                    emit_s(nxt, kb)
                # one fp8 DoubleRow AV matmul per slot (2 k-subtiles each):
                # head j = kb%2 over token-block pair kb2 = kb//2
                av_mm(kb % 2, kb // 2)
                # pop fillers: enough each slot to drain the queue by unit end
                npop = -(-len(ops) // (TB - kb))
                for _ in range(min(npop, 3)):
                    if ops:
                        ops.pop(0)[1]()
            for kind, op in ops:
                op()
            et_store.pop(u)

            # epilogue: drain AV psum to SBUF (frees banks), reciprocal of
            # the ones-row, broadcast via DRAM round-trip, normalize on Pool
            stg = stgp.tile([65, 2 * CH], f32, tag="stg", name=f"stg{hp}_{cc}")
            sums = recp.tile([1, 2 * CH], f32, tag="sums", name=f"sums{hp}_{cc}")
            nc.vector.tensor_copy(out=stg[:, 0 * CH:1 * CH], in_=avs[0][:65, :])
            nc.scalar.copy(out=stg[:, 1 * CH:2 * CH], in_=avs[1][:65, :])
            nc.vector.tensor_copy(out=sums[:, 0 * CH:1 * CH], in_=avs[0][64:65, :])
            nc.vector.tensor_copy(out=sums[:, 1 * CH:2 * CH], in_=avs[1][64:65, :])
            rec = recp.tile([1, 2 * CH], f32, tag="rec", name=f"rec{hp}_{cc}")
            nc.vector.reciprocal_approx_fast(out=rec, in_=sums)
            nc.gpsimd.dma_start(out=rscr.ap()[hp, cc], in_=rec)
            rec64 = recp.tile([64, 2 * CH], f32, tag="rec64", name=f"rb{hp}_{cc}")
            nc.gpsimd.dma_start(
                out=rec64, in_=rscr.ap()[hp, cc].partition_broadcast(64)
            )
            for j, eng in ((0, nc.gpsimd), (1, nc.vector)):
                h = 2 * hp + j
                eng.tensor_mul(
                    out=ot_sb[h // 2][cc][(h % 2) * 64:(h % 2) * 64 + 64, :],
                    in0=stg[0:64, j * CH:(j + 1) * CH],
                    in1=rec64[:, j * CH:(j + 1) * CH],
                )
    nc.finalize()
    return nc


def _in_maps(x, qkv_w, qkv_b, proj_w):
    import ml_dtypes

    bf = ml_dtypes.bfloat16
    maps = []
    for c in range(NCORE):
        b, hg = c // 4, c % 4
        fs = slice(hg * F, (hg + 1) * F)
        q, k = qkv_w[fs], qkv_w[DIM:][fs]
        # device column order [k01 | q01 | k23 | q23]
        wqk = np.concatenate([k[:128], q[:128], k[128:], q[128:]], 0)  # [512,1024]
        bqk = np.concatenate([qkv_b[fs], qkv_b[DIM:][fs]], 0)
        maps.append({
            "xt": np.ascontiguousarray(x[b].T).astype(bf),
            "wqk": np.ascontiguousarray(wqk.T).astype(bf),
            "wv": np.ascontiguousarray(qkv_w[2 * DIM:][fs].T).astype(bf),
            "bqk": np.ascontiguousarray(bqk),
            "bv": np.ascontiguousarray(qkv_b[2 * DIM:][fs]),
            "pw": np.ascontiguousarray(proj_w[:, fs].T).astype(bf),
        })
    return maps


def _run(inputs, trace=False, trace_kwargs=None):
    from concourse.bass_utils import run_bass_kernel_spmd

    if "nc" not in _cache:
        _cache["nc"] = _build()
    nc = _cache["nc"]
    maps = _in_maps(inputs["x"], inputs["qkv_w"], inputs["qkv_b"], inputs["proj_w"])
    res = run_bass_kernel_spmd(
        nc, maps, list(range(NCORE)), trace=trace, **(trace_kwargs or {})
    )
    outs = [r["out"] for r in res.results]              # [1024, 2048] bf16 partials
    full = np.empty((B, N, DIM), dtype=np.float32)
    for b in range(B):
        acc = outs[4 * b].astype(np.float32)
        for c in range(4 * b + 1, 4 * b + 4):
            acc += outs[c].astype(np.float32)
        full[b] = acc.T + inputs["proj_b"]
    return full, res


def kernel(**inputs) -> np.ndarray:
    out, _ = _run(inputs, trace=False)
    return out
